# revision 38
# baseline (speedup 1.0000x reference)
"""Multi-head graph attention (GAT-style) Trainium2 Bass kernel, v3.

Full-input contract: kernel(**inputs) takes the complete arrays, shards
batch-wise across 8 NeuronCores (2 batches each), and gathers the output.

Math per batch b, head h (KD=16 head dim):
  Q = h @ Wq_h, K = h @ Wk_h, V = h @ Wv_h            [N, 16]
  compatT[m, n] = (K Q^T)[m, n]                        [N, N] (transposed)
  p = exp(0.25 * compatT) * adjT                       (mask after exp; exact:
      masked entries are exactly 0, matching softmax(-inf) * adj)
  headsT[v, n] = (V'.T @ p)  with V' = [V | 1 | 0pad]  -> row 16 = denominator
  out[n, :] = sum_h (headsT_h / den_h).T @ Wout_h + h[n, :]

Design (v3): ACT's exp stream (~141us) is the pacing engine; everything
else hides under it.
 - Heads live in 32-partition bands (head h -> quad h//4, band h%4).
   Banded zero-padded projection weights put q/k of band b at SBUF
   partitions [32b, 32b+16), so the K=16 compat matmuls of a head PAIR
   run concurrently via PE row tiling (tile_position=(32b, 0)).
 - Each pair's two [128,512] compat tiles share one 2-bank PSUM buffer;
   one exp call covers both.  Pair buffers rotate 3-deep (banks 0-5);
   PV accumulators for the two quads hold banks 6-7 per n-half.
 - PV matmuls lag one pair behind compat in PE program order.
 - The per-n-half epilogue (hu copy, reciprocal of the denominator rows
   via a [128,16] reshape bounce through DRAM, band-broadcast, W_out
   matmul + residual) is software-pipelined INTO the next n-half's
   stream: DVE/DMA work at step 2, the PE out-matmuls at step 16, so
   the in-order PE queue never blocks the next compat matmuls.
 - Next batch's input DMAs prefetch at the previous batch's nt=0 start;
   its projection/V-build units spread one-per-step through nt=1.
"""

import os
import numpy as np
import ml_dtypes
from contextlib import ExitStack

import concourse.bass as bass
import concourse.mybir as mybir
import concourse.tile as tile
from concourse.bass_utils import run_bass_kernel_spmd

B, N, E, H, KD = 16, 1024, 128, 8, 16
CORES = 8
BPC = B // CORES  # batches per core
F32 = mybir.dt.float32
BF16 = mybir.dt.bfloat16
NT = 512  # n-half width (one PSUM bank of fp32 per [128, NT] tile)
MC = N // 128  # number of 128-row chunks of m
VP = 32  # padded per-head V columns (16 vals + 1 ones + 15 zeros)
NR = NT // 32  # free elems per lane in the [128, NR] reciprocal

EPI1_STEP = int(os.environ.get("KEPI1", "2"))
EPI2_STEP = int(os.environ.get("KEPI2", "16"))


def build_kernel():
    nc = bass.Bass()
    hT_d = nc.dram_tensor("ht", [BPC, E, N], BF16, kind="ExternalInput")
    h_d = nc.dram_tensor("hn", [BPC, N, E], F32, kind="ExternalInput")
    adjt_d = nc.dram_tensor("adjt", [BPC, N, N], BF16, kind="ExternalInput")
    wqb_d = nc.dram_tensor("wqb", [2, E, 128], BF16, kind="ExternalInput")
    wkb_d = nc.dram_tensor("wkb", [2, E, 128], BF16, kind="ExternalInput")
    wv_d = nc.dram_tensor("wv", [E, H * KD], BF16, kind="ExternalInput")
    wob_d = nc.dram_tensor("wob", [2, 128, E], BF16, kind="ExternalInput")
    sel_d = nc.dram_tensor("sel", [4, 128], F32, kind="ExternalInput")
    out_d = nc.dram_tensor("out", [BPC, N, E], F32, kind="ExternalOutput")

    with ExitStack() as ctx:
        tc = ctx.enter_context(tile.TileContext(nc))
        consts = ctx.enter_context(tc.tile_pool(name="consts", bufs=1))
        io_pool = ctx.enter_context(tc.tile_pool(name="io", bufs=2))
        qk_pool = ctx.enter_context(tc.tile_pool(name="qk", bufs=2))
        v_pool = ctx.enter_context(tc.tile_pool(name="v", bufs=2))
        pe_pool = ctx.enter_context(tc.tile_pool(name="pe", bufs=4))
        pm_pool = ctx.enter_context(tc.tile_pool(name="pm", bufs=4))
        hn_pool = ctx.enter_context(tc.tile_pool(name="hd", bufs=2))
        dn_pool = ctx.enter_context(tc.tile_pool(name="dn", bufs=2))
        ob_pool = ctx.enter_context(tc.tile_pool(name="ob", bufs=2))
        ps_pair = ctx.enter_context(tc.tile_pool(name="ps_p", bufs=3, space="PSUM"))
        ps_hp = ctx.enter_context(tc.tile_pool(name="ps_h", bufs=1, space="PSUM"))
        dram = ctx.enter_context(tc.tile_pool(name="dram", bufs=2, space="DRAM"))

        wv_sb = consts.tile([E, H * KD], BF16, tag="wv")
        wq_q = [consts.tile([E, 128], BF16, tag=f"wq{q}", name=f"wq{q}") for q in range(2)]
        wk_q = [consts.tile([E, 128], BF16, tag=f"wk{q}", name=f"wk{q}") for q in range(2)]
        wo_q = [consts.tile([128, E], BF16, tag=f"wo{q}", name=f"wo{q}") for q in range(2)]

        def load_weights():
            for q in range(2):
                nc.sync.dma_start(out=wq_q[q], in_=wqb_d[q])
                nc.sync.dma_start(out=wk_q[q], in_=wkb_d[q])
                nc.sync.dma_start(out=wo_q[q], in_=wob_d[q])
            nc.sync.dma_start(out=wv_sb, in_=wv_d[:, :])
            nc.sync.dma_start(out=sel_sb, in_=sel_d[:, :])

        # band-broadcast selector: sel[c, 32c+j] = 1 (j < 17) so
        # (sel.T @ rec4) replicates rec4 row c into band c's rows on PE.
        sel_sb = consts.tile([4, 128], F32, tag="sel")

        ios = {}

        def prefetch(b, mid=None):
            hT_sb = io_pool.tile([E, N], BF16, tag="ht", name="hts")
            nc.sync.dma_start(out=hT_sb, in_=hT_d[b, :, :])
            if mid is not None:
                mid()
            adjT_sb = io_pool.tile([128, MC, N], BF16, tag="adj", name="adjs")
            nc.sync.dma_start(
                out=adjT_sb, in_=adjt_d[b].rearrange("(c p) n -> p c n", p=128)
            )
            h_sb = io_pool.tile([128, MC, E], F32, tag="hn", name="hns")
            nc.sync.dma_start(
                out=h_sb, in_=h_d[b].rearrange("(c p) e -> p c e", p=128)
            )
            ios[b] = (hT_sb, adjT_sb, h_sb)

        bands = {}

        def make_prologue_units(b):
            """Projection + V-build for batch b as a list of closures,
            each one PE matmul + one DVE copy (+memsets)."""
            hT_sb = ios[b][0]
            qb = [qk_pool.tile([128, N], BF16, tag=f"q{q}", name=f"qb{q}") for q in range(2)]
            kb = [qk_pool.tile([128, N], BF16, tag=f"k{q}", name=f"kb{q}") for q in range(2)]
            v_nat = [
                v_pool.tile([128, H, VP], BF16, tag=f"v{mc}", name=f"v{mc}")
                for mc in range(MC)
            ]
            bands[b] = (qb, kb, v_nat)
            units = []

            def proj_unit(w_sb, dst, nt):
                def run():
                    ps = ps_pair.tile([128, NT], F32, tag="pair", name="pp")
                    nc.tensor.matmul(
                        out=ps,
                        lhsT=w_sb,
                        rhs=hT_sb[:, nt * NT : (nt + 1) * NT],
                        start=True,
                        stop=True,
                    )
                    nc.vector.tensor_copy(
                        out=dst[:, nt * NT : (nt + 1) * NT], in_=ps
                    )

                return run

            def v_unit(mc):
                def run():
                    v_ps = ps_pair.tile([128, H * KD], F32, tag="pair", name="vp")
                    nc.tensor.matmul(
                        out=v_ps,
                        lhsT=hT_sb[:, mc * 128 : (mc + 1) * 128],
                        rhs=wv_sb,
                        start=True,
                        stop=True,
                    )
                    vt = v_nat[mc]
                    nc.vector.tensor_copy(
                        out=vt[:, :, 0:KD],
                        in_=v_ps.rearrange("p (h k) -> p h k", k=KD),
                    )
                    nc.vector.memset(vt[:, :, KD : KD + 1], 1.0)
                    nc.vector.memset(vt[:, :, KD + 1 : VP], 0.0)

                return run

            # streaming order: first the k/q halves the next segment's early
            # steps need, then V tiles in PV-consumption order, then the
            # q n-halves only needed by the following (nt=1) segment.
            units.append(proj_unit(wk_q[0], kb[0], 0))
            units.append(proj_unit(wq_q[0], qb[0], 0))
            units.append(proj_unit(wk_q[1], kb[1], 0))
            units.append(proj_unit(wq_q[1], qb[1], 0))
            units.append(v_unit(0))
            units.append(v_unit(1))
            units.append(v_unit(2))
            units.append(proj_unit(wk_q[0], kb[0], 1))
            units.append(proj_unit(wk_q[1], kb[1], 1))
            for mc in range(3, MC):
                units.append(v_unit(mc))
            units.append(proj_unit(wq_q[0], qb[0], 1))
            units.append(proj_unit(wq_q[1], qb[1], 1))
            return units

        def make_epilogue(b, nt, hp):
            """Normalize + W_out + residual for (b, nt) as staged closures:
            each stage's DVE ops run only after their DMA deps have had a
            few pipeline steps of latency, so the in-order DVE queue never
            blocks the exp/mask stream."""
            h_sb = ios[b][2]
            hus, r128s, hn2 = [], [], []
            rec_drams = []

            rec4s, rec_ps = [], []

            def stage_a():  # hu copies + [4,NT]->[128,NR] gather DMAs
                for q in range(2):
                    hu = hn_pool.tile([128, NT], F32, tag=f"hu{q}", name=f"hu{q}")
                    nc.vector.tensor_copy(out=hu, in_=hp[q])
                    hus.append(hu)
                for q in range(2):
                    d128 = dn_pool.tile([128, NR], F32, tag=f"d128{q}", name=f"d1{q}")
                    for c in range(4):
                        src = hus[q][32 * c + KD : 32 * c + KD + 1, :]
                        src_r = bass.AP(
                            tensor=src.tensor,
                            offset=src.offset,
                            ap=[list(src.ap[0]), [NR, 32], [1, NR]],
                        )
                        nc.gpsimd.dma_start(
                            out=d128[32 * c : 32 * c + 32, :], in_=src_r
                        )
                    r128s.append(d128)

            def stage_b():  # reciprocal + scatter back to [4, NT] rows
                for q in range(2):
                    r128 = dn_pool.tile([128, NR], F32, tag=f"r128{q}", name=f"r1{q}")
                    nc.vector.reciprocal(out=r128, in_=r128s[q])
                    r128s[q] = r128
                for q in range(2):
                    rec4 = dn_pool.tile([4, NT], F32, tag=f"rec4{q}", name=f"rc{q}")
                    dst = rec4[:, :]
                    dst_r = bass.AP(
                        tensor=dst.tensor,
                        offset=dst.offset,
                        ap=[list(dst.ap[0]), [NR, 32], [1, NR]],
                    )
                    nc.gpsimd.dma_start(out=dst_r, in_=r128s[q])
                    rec4s.append(rec4)

            def stage_c():  # selector-matmul broadcast on PE
                for q in range(2):
                    bc_ps = ps_pair.tile([128, NT], F32, tag="pair", name="bc")
                    nc.tensor.matmul(
                        out=bc_ps, lhsT=sel_sb, rhs=rec4s[q], start=True, stop=True
                    )
                    rec_ps.append(bc_ps)

            def stage_c2():  # normalize muls
                for q in range(2):
                    hn = hn_pool.tile([128, NT], BF16, tag=f"hn{q}", name=f"hn{q}")
                    nc.vector.tensor_mul(hn, hus[q], rec_ps[q])
                    hn2.append(hn)

            def stage_d():  # W_out matmuls + residual + store (paired chunks)
                for cp in range(NT // 256):
                    cc = nt * (NT // 128) + cp * 2
                    o_ps = ps_pair.tile([128, 2, E], F32, tag="pair", name="op")
                    for cl in range(2):
                        for q in range(2):
                            nc.tensor.matmul(
                                out=o_ps[:, cl, :],
                                lhsT=hn2[q][
                                    :, (cp * 2 + cl) * 128 : (cp * 2 + cl + 1) * 128
                                ],
                                rhs=wo_q[q],
                                start=(q == 0),
                                stop=(q == 1),
                            )
                    ob = ob_pool.tile([128, 2, E], F32, tag="ob", name="ob")
                    nc.vector.tensor_add(ob, o_ps, h_sb[:, cc : cc + 2, :])
                    nc.sync.dma_start(
                        out=out_d[b, cc * 128 : (cc + 2) * 128, :].rearrange(
                            "(c p) e -> p c e", p=128
                        ),
                        in_=ob,
                    )

            return [
                (1, stage_a),
                (6, stage_b),
                (10, stage_c),
                (13, stage_c2),
                (17, stage_d),
            ]

        # ---- main stream over (batch, n-half) segments ----
        prefetch(0, mid=load_weights)
        b0_units = make_prologue_units(0)
        for u in b0_units[:4]:
            u()
        pending = None
        prologue_units = b0_units[4:]
        for b in range(BPC):
            qb, kb, v_nat = bands.pop(b)
            for nt in range(N // NT):
                if nt == 0 and b + 1 < BPC:
                    prefetch(b + 1)
                if nt == 1 and b + 1 < BPC:
                    prologue_units = make_prologue_units(b + 1)
                hp = [
                    ps_hp.tile([128, NT], F32, tag=f"hp{q}", name=f"hp{q}")
                    for q in range(2)
                ]
                prev = None

                def emit_pv(pm, mc, pair):
                    for j in range(2):
                        hh = pair * 2 + j
                        c = hh % 4
                        nc.tensor.matmul(
                            out=hp[hh // 4][32 * c : 32 * c + VP, :],
                            lhsT=v_nat[mc][:, hh, :],
                            rhs=pm[:, j * NT : (j + 1) * NT],
                            start=(mc == 0),
                            stop=(mc == MC - 1),
                            tile_position=(0, 32 * c),
                        )

                step = 0
                for mc in range(MC):
                    for pair in range(4):
                        quad, b0 = pair // 2, (pair * 2) % 4
                        ps = ps_pair.tile([128, 2 * NT], F32, tag="pair", name="cp")
                        for j in range(2):
                            bd = b0 + j
                            nc.tensor.matmul(
                                out=ps[:, j * NT : (j + 1) * NT],
                                lhsT=kb[quad][
                                    32 * bd : 32 * bd + KD,
                                    mc * 128 : (mc + 1) * 128,
                                ],
                                rhs=qb[quad][
                                    32 * bd : 32 * bd + KD,
                                    nt * NT : (nt + 1) * NT,
                                ],
                                start=True,
                                stop=True,
                                tile_position=(32 * bd, 0),
                            )
                        if prev is not None:
                            emit_pv(*prev)
                        stage_fired = False
                        if pending is not None and pending and step == pending[0][0]:
                            pending.pop(0)[1]()
                            stage_fired = True
                            if not pending:
                                pending = None
                        if not stage_fired and prologue_units and (
                            step >= 4 or (b == 0 and nt == 0)
                        ):
                            prologue_units.pop(0)()
                        p_sb = pe_pool.tile([128, 2 * NT], BF16, tag="p", name="p")
                        nc.scalar.activation(
                            out=p_sb,
                            in_=ps,
                            func=mybir.ActivationFunctionType.Exp,
                            scale=0.25,
                        )
                        pm = pm_pool.tile([128, 2 * NT], BF16, tag="pm", name="pm")
                        adj_src = adjT_sb_of(ios, b)[:, mc, nt * NT : (nt + 1) * NT]
                        adj_rep = bass.AP(
                            tensor=adj_src.tensor,
                            offset=adj_src.offset,
                            ap=[list(adj_src.ap[0]), [0, 2]]
                            + [list(a) for a in adj_src.ap[1:]],
                        )
                        nc.vector.tensor_mul(pm, p_sb, adj_rep)
                        prev = (pm, mc, pair)
                        step += 1
                emit_pv(*prev)
                pending = make_epilogue(b, nt, hp)
            del ios[b]
        for _, fn in pending:
            fn()
    return nc


def adjT_sb_of(ios, b):
    return ios[b][1]


def _split_multi_waits(nc):
    """walrus codegen in this container allows only one sync-wait per
    instruction; hoist extra waits onto preceding same-engine nops."""
    import copy
    import bass_rust

    tmpl_nc = bass.Bass()
    tmpls = {}
    for en in ["vector", "scalar", "tensor", "gpsimd", "sync"]:
        ins = getattr(tmpl_nc, en).nop().ins
        tmpls[str(ins.engine)] = ins

    uid = [0]
    for fn in nc.m.functions:
        for bb in fn.blocks:
            out = []
            for ins in bb.instructions:
                si = ins.sync_info
                waits = list(si.on_wait) if si is not None else []
                if len(waits) > 1:
                    for w in waits[:-1]:
                        nop = copy.deepcopy(tmpls[str(ins.engine)])
                        uid[0] += 1
                        nop.name = f"I-splitw-{uid[0]}"
                        nop.sync_info = bass_rust.SyncInfo(
                            on_wait=[w], on_update=[]
                        )
                        out.append(nop)
                    ins.sync_info = bass_rust.SyncInfo(
                        on_wait=[waits[-1]], on_update=list(si.on_update)
                    )
                out.append(ins)
            bb.instructions = out
    return nc


_cache = {}


def _get_nc():
    if "nc" not in _cache:
        _cache["nc"] = _split_multi_waits(build_kernel())
    return _cache["nc"]


def _prep_weights(W_query, W_key, W_val, W_out):
    bf = ml_dtypes.bfloat16
    wqb = np.zeros((2, E, 128), bf)
    wkb = np.zeros((2, E, 128), bf)
    wob = np.zeros((2, 128, E), bf)
    for h in range(H):
        q, c = h // 4, h % 4
        wqb[q, :, 32 * c : 32 * c + KD] = W_query[h].astype(bf)
        wkb[q, :, 32 * c : 32 * c + KD] = W_key[h].astype(bf)
        wob[q, 32 * c : 32 * c + KD, :] = W_out[h].astype(bf)
    wv = np.ascontiguousarray(
        np.asarray(W_val, np.float32).transpose(1, 0, 2).reshape(E, H * KD)
    ).astype(bf)
    sel = np.zeros((4, 128), np.float32)
    for c in range(4):
        sel[c, 32 * c : 32 * c + KD + 1] = 1.0
    return wqb, wkb, wv, wob, sel


def kernel(h, adj_c, W_query, W_key, W_val, W_out, trace=False):
    h = np.asarray(h, np.float32)
    adj = np.asarray(adj_c)
    bf = ml_dtypes.bfloat16
    hT = np.ascontiguousarray(h.transpose(0, 2, 1)).astype(bf)  # [B, E, N]
    adjT = np.ascontiguousarray(
        adj.transpose(0, 2, 1).astype(bf)
    )  # [B, N(m), N(n)] bf16
    wqb, wkb, wv, wob, sel = _prep_weights(
        np.asarray(W_query, np.float32),
        np.asarray(W_key, np.float32),
        np.asarray(W_val, np.float32),
        np.asarray(W_out, np.float32),
    )

    nc = _get_nc()
    in_maps = []
    for c in range(CORES):
        s = slice(c * BPC, (c + 1) * BPC)
        in_maps.append(
            {
                "ht": np.ascontiguousarray(hT[s]),
                "hn": np.ascontiguousarray(h[s]),
                "adjt": np.ascontiguousarray(adjT[s]),
                "wqb": wqb,
                "wkb": wkb,
                "wv": wv,
                "wob": wob,
                "sel": sel,
            }
        )
    res = run_bass_kernel_spmd(nc, in_maps, core_ids=list(range(CORES)), trace=trace)
    out = np.concatenate([r["out"] for r in res.results], axis=0)
    if trace:
        return out, res
    return out


# revision 39
# speedup vs baseline: 1.0005x; 1.0005x over previous
"""Multi-head graph attention (GAT-style) Trainium2 Bass kernel, v3.

Full-input contract: kernel(**inputs) takes the complete arrays, shards
batch-wise across 8 NeuronCores (2 batches each), and gathers the output.

Math per batch b, head h (KD=16 head dim):
  Q = h @ Wq_h, K = h @ Wk_h, V = h @ Wv_h            [N, 16]
  compatT[m, n] = (K Q^T)[m, n]                        [N, N] (transposed)
  p = exp(0.25 * compatT) * adjT                       (mask after exp; exact:
      masked entries are exactly 0, matching softmax(-inf) * adj)
  headsT[v, n] = (V'.T @ p)  with V' = [V | 1 | 0pad]  -> row 16 = denominator
  out[n, :] = sum_h (headsT_h / den_h).T @ Wout_h + h[n, :]

Design (v3): ACT's exp stream (~141us) is the pacing engine; everything
else hides under it.
 - Heads live in 32-partition bands (head h -> quad h//4, band h%4).
   Banded zero-padded projection weights put q/k of band b at SBUF
   partitions [32b, 32b+16), so the K=16 compat matmuls of a head PAIR
   run concurrently via PE row tiling (tile_position=(32b, 0)).
 - Each pair's two [128,512] compat tiles share one 2-bank PSUM buffer;
   one exp call covers both.  Pair buffers rotate 3-deep (banks 0-5);
   PV accumulators for the two quads hold banks 6-7 per n-half.
 - PV matmuls lag one pair behind compat in PE program order.
 - The per-n-half epilogue (hu copy, reciprocal of the denominator rows
   via a [128,16] reshape bounce through DRAM, band-broadcast, W_out
   matmul + residual) is software-pipelined INTO the next n-half's
   stream: DVE/DMA work at step 2, the PE out-matmuls at step 16, so
   the in-order PE queue never blocks the next compat matmuls.
 - Next batch's input DMAs prefetch at the previous batch's nt=0 start;
   its projection/V-build units spread one-per-step through nt=1.
"""

import os
import numpy as np
import ml_dtypes
from contextlib import ExitStack

import concourse.bass as bass
import concourse.mybir as mybir
import concourse.tile as tile
from concourse.bass_utils import run_bass_kernel_spmd

B, N, E, H, KD = 16, 1024, 128, 8, 16
CORES = 8
BPC = B // CORES  # batches per core
F32 = mybir.dt.float32
BF16 = mybir.dt.bfloat16
NT = 512  # n-half width (one PSUM bank of fp32 per [128, NT] tile)
MC = N // 128  # number of 128-row chunks of m
VP = 32  # padded per-head V columns (16 vals + 1 ones + 15 zeros)
NR = NT // 32  # free elems per lane in the [128, NR] reciprocal

EPI1_STEP = int(os.environ.get("KEPI1", "2"))
EPI2_STEP = int(os.environ.get("KEPI2", "16"))


def build_kernel():
    nc = bass.Bass()
    hT_d = nc.dram_tensor("ht", [BPC, E, N], BF16, kind="ExternalInput")
    h_d = nc.dram_tensor("hn", [BPC, N, E], F32, kind="ExternalInput")
    adjt_d = nc.dram_tensor("adjt", [BPC, N, N], BF16, kind="ExternalInput")
    wqb_d = nc.dram_tensor("wqb", [2, E, 128], BF16, kind="ExternalInput")
    wkb_d = nc.dram_tensor("wkb", [2, E, 128], BF16, kind="ExternalInput")
    wv_d = nc.dram_tensor("wv", [E, H * KD], BF16, kind="ExternalInput")
    wob_d = nc.dram_tensor("wob", [2, 128, E], BF16, kind="ExternalInput")
    sel_d = nc.dram_tensor("sel", [4, 128], F32, kind="ExternalInput")
    out_d = nc.dram_tensor("out", [BPC, N, E], F32, kind="ExternalOutput")

    with ExitStack() as ctx:
        tc = ctx.enter_context(tile.TileContext(nc))
        consts = ctx.enter_context(tc.tile_pool(name="consts", bufs=1))
        io_pool = ctx.enter_context(tc.tile_pool(name="io", bufs=2))
        qk_pool = ctx.enter_context(tc.tile_pool(name="qk", bufs=2))
        v_pool = ctx.enter_context(tc.tile_pool(name="v", bufs=2))
        pe_pool = ctx.enter_context(tc.tile_pool(name="pe", bufs=4))
        pm_pool = ctx.enter_context(tc.tile_pool(name="pm", bufs=4))
        hn_pool = ctx.enter_context(tc.tile_pool(name="hd", bufs=2))
        dn_pool = ctx.enter_context(tc.tile_pool(name="dn", bufs=2))
        ob_pool = ctx.enter_context(tc.tile_pool(name="ob", bufs=2))
        ps_pair = ctx.enter_context(tc.tile_pool(name="ps_p", bufs=3, space="PSUM"))
        ps_hp = ctx.enter_context(tc.tile_pool(name="ps_h", bufs=1, space="PSUM"))
        dram = ctx.enter_context(tc.tile_pool(name="dram", bufs=2, space="DRAM"))

        wv_sb = consts.tile([E, H * KD], BF16, tag="wv")
        wq_q = [consts.tile([E, 128], BF16, tag=f"wq{q}", name=f"wq{q}") for q in range(2)]
        wk_q = [consts.tile([E, 128], BF16, tag=f"wk{q}", name=f"wk{q}") for q in range(2)]
        wo_q = [consts.tile([128, E], BF16, tag=f"wo{q}", name=f"wo{q}") for q in range(2)]

        def load_weights():
            for q in range(2):
                nc.sync.dma_start(out=wq_q[q], in_=wqb_d[q])
                nc.sync.dma_start(out=wk_q[q], in_=wkb_d[q])
                nc.sync.dma_start(out=wo_q[q], in_=wob_d[q])
            nc.sync.dma_start(out=wv_sb, in_=wv_d[:, :])
            nc.sync.dma_start(out=sel_sb, in_=sel_d[:, :])

        # band-broadcast selector: sel[c, 32c+j] = 1 (j < 17) so
        # (sel.T @ rec4) replicates rec4 row c into band c's rows on PE.
        sel_sb = consts.tile([4, 128], F32, tag="sel")

        ios = {}

        def prefetch(b, mid=None):
            hT_sb = io_pool.tile([E, N], BF16, tag="ht", name="hts")
            nc.sync.dma_start(out=hT_sb, in_=hT_d[b, :, :])
            if mid is not None:
                mid()
            adjT_sb = io_pool.tile([128, MC, N], BF16, tag="adj", name="adjs")
            nc.sync.dma_start(
                out=adjT_sb, in_=adjt_d[b].rearrange("(c p) n -> p c n", p=128)
            )
            h_sb = io_pool.tile([128, MC, E], F32, tag="hn", name="hns")
            nc.sync.dma_start(
                out=h_sb, in_=h_d[b].rearrange("(c p) e -> p c e", p=128)
            )
            ios[b] = (hT_sb, adjT_sb, h_sb)

        bands = {}

        def make_prologue_units(b):
            """Projection + V-build for batch b as a list of closures,
            each one PE matmul + one DVE copy (+memsets)."""
            hT_sb = ios[b][0]
            qb = [qk_pool.tile([128, N], BF16, tag=f"q{q}", name=f"qb{q}") for q in range(2)]
            kb = [qk_pool.tile([128, N], BF16, tag=f"k{q}", name=f"kb{q}") for q in range(2)]
            v_nat = [
                v_pool.tile([128, H, VP], BF16, tag=f"v{mc}", name=f"v{mc}")
                for mc in range(MC)
            ]
            bands[b] = (qb, kb, v_nat)
            units = []

            def proj_unit(w_sb, dst, nt):
                def run():
                    ps = ps_pair.tile([128, NT], F32, tag="pair", name="pp")
                    nc.tensor.matmul(
                        out=ps,
                        lhsT=w_sb,
                        rhs=hT_sb[:, nt * NT : (nt + 1) * NT],
                        start=True,
                        stop=True,
                    )
                    nc.vector.tensor_copy(
                        out=dst[:, nt * NT : (nt + 1) * NT], in_=ps
                    )

                return run

            def v_unit(mc):
                def run():
                    v_ps = ps_pair.tile([128, H * KD], F32, tag="pair", name="vp")
                    nc.tensor.matmul(
                        out=v_ps,
                        lhsT=hT_sb[:, mc * 128 : (mc + 1) * 128],
                        rhs=wv_sb,
                        start=True,
                        stop=True,
                    )
                    vt = v_nat[mc]
                    nc.vector.tensor_copy(
                        out=vt[:, :, 0:KD],
                        in_=v_ps.rearrange("p (h k) -> p h k", k=KD),
                    )
                    nc.vector.memset(vt[:, :, KD : KD + 1], 1.0)
                    nc.vector.memset(vt[:, :, KD + 1 : VP], 0.0)

                return run

            # streaming order: first the k/q halves the next segment's early
            # steps need, then V tiles in PV-consumption order, then the
            # q n-halves only needed by the following (nt=1) segment.
            units.append(proj_unit(wk_q[0], kb[0], 0))
            units.append(proj_unit(wq_q[0], qb[0], 0))
            units.append(proj_unit(wk_q[1], kb[1], 0))
            units.append(proj_unit(wq_q[1], qb[1], 0))
            units.append(v_unit(0))
            units.append(v_unit(1))
            units.append(v_unit(2))
            units.append(proj_unit(wk_q[0], kb[0], 1))
            units.append(proj_unit(wk_q[1], kb[1], 1))
            for mc in range(3, MC):
                units.append(v_unit(mc))
            units.append(proj_unit(wq_q[0], qb[0], 1))
            units.append(proj_unit(wq_q[1], qb[1], 1))
            return units

        def make_epilogue(b, nt, hp):
            """Normalize + W_out + residual for (b, nt) as staged closures:
            each stage's DVE ops run only after their DMA deps have had a
            few pipeline steps of latency, so the in-order DVE queue never
            blocks the exp/mask stream."""
            h_sb = ios[b][2]
            hus, r128s, hn2 = [], [], []
            rec_drams = []

            rec4s, rec_ps = [], []

            def stage_a():  # hu copies + [4,NT]->[128,NR] gather DMAs
                for q in range(2):
                    hu = hn_pool.tile([128, NT], F32, tag=f"hu{q}", name=f"hu{q}")
                    nc.vector.tensor_copy(out=hu, in_=hp[q])
                    hus.append(hu)
                for q in range(2):
                    d128 = dn_pool.tile([128, NR], F32, tag=f"d128{q}", name=f"d1{q}")
                    for c in range(4):
                        src = hus[q][32 * c + KD : 32 * c + KD + 1, :]
                        src_r = bass.AP(
                            tensor=src.tensor,
                            offset=src.offset,
                            ap=[list(src.ap[0]), [NR, 32], [1, NR]],
                        )
                        nc.gpsimd.dma_start(
                            out=d128[32 * c : 32 * c + 32, :], in_=src_r
                        )
                    r128s.append(d128)

            def stage_b():  # reciprocal + scatter back to [4, NT] rows
                for q in range(2):
                    r128 = dn_pool.tile([128, NR], F32, tag=f"r128{q}", name=f"r1{q}")
                    nc.vector.reciprocal(out=r128, in_=r128s[q])
                    r128s[q] = r128
                for q in range(2):
                    rec4 = dn_pool.tile([4, NT], F32, tag=f"rec4{q}", name=f"rc{q}")
                    dst = rec4[:, :]
                    dst_r = bass.AP(
                        tensor=dst.tensor,
                        offset=dst.offset,
                        ap=[list(dst.ap[0]), [NR, 32], [1, NR]],
                    )
                    nc.gpsimd.dma_start(out=dst_r, in_=r128s[q])
                    rec4s.append(rec4)

            def stage_c():  # selector-matmul broadcast on PE
                for q in range(2):
                    bc_ps = ps_pair.tile([128, NT], F32, tag="pair", name="bc")
                    nc.tensor.matmul(
                        out=bc_ps, lhsT=sel_sb, rhs=rec4s[q], start=True, stop=True
                    )
                    rec_ps.append(bc_ps)

            def stage_c2():  # normalize muls
                for q in range(2):
                    hn = hn_pool.tile([128, NT], BF16, tag=f"hn{q}", name=f"hn{q}")
                    nc.vector.tensor_mul(hn, hus[q], rec_ps[q])
                    hn2.append(hn)

            def stage_d():  # W_out matmuls + residual + store (paired chunks)
                for cp in range(NT // 256):
                    cc = nt * (NT // 128) + cp * 2
                    o_ps = ps_pair.tile([128, 2, E], F32, tag="pair", name="op")
                    for cl in range(2):
                        for q in range(2):
                            nc.tensor.matmul(
                                out=o_ps[:, cl, :],
                                lhsT=hn2[q][
                                    :, (cp * 2 + cl) * 128 : (cp * 2 + cl + 1) * 128
                                ],
                                rhs=wo_q[q],
                                start=(q == 0),
                                stop=(q == 1),
                            )
                    ob = ob_pool.tile([128, 2, E], F32, tag="ob", name="ob")
                    nc.vector.tensor_add(ob, o_ps, h_sb[:, cc : cc + 2, :])
                    nc.sync.dma_start(
                        out=out_d[b, cc * 128 : (cc + 2) * 128, :].rearrange(
                            "(c p) e -> p c e", p=128
                        ),
                        in_=ob,
                    )

            return [
                (1, stage_a),
                (6, stage_b),
                (13, stage_c),
                (16, stage_c2),
                (20, stage_d),
            ]

        # ---- main stream over (batch, n-half) segments ----
        prefetch(0, mid=load_weights)
        b0_units = make_prologue_units(0)
        for u in b0_units[:4]:
            u()
        pending = None
        prologue_units = b0_units[4:]
        for b in range(BPC):
            qb, kb, v_nat = bands.pop(b)
            for nt in range(N // NT):
                if nt == 0 and b + 1 < BPC:
                    prefetch(b + 1)
                if nt == 1 and b + 1 < BPC:
                    prologue_units = make_prologue_units(b + 1)
                hp = [
                    ps_hp.tile([128, NT], F32, tag=f"hp{q}", name=f"hp{q}")
                    for q in range(2)
                ]
                prev = None

                def emit_pv(pm, mc, pair):
                    for j in range(2):
                        hh = pair * 2 + j
                        c = hh % 4
                        nc.tensor.matmul(
                            out=hp[hh // 4][32 * c : 32 * c + VP, :],
                            lhsT=v_nat[mc][:, hh, :],
                            rhs=pm[:, j * NT : (j + 1) * NT],
                            start=(mc == 0),
                            stop=(mc == MC - 1),
                            tile_position=(0, 32 * c),
                        )

                step = 0
                for mc in range(MC):
                    for pair in range(4):
                        quad, b0 = pair // 2, (pair * 2) % 4
                        ps = ps_pair.tile([128, 2 * NT], F32, tag="pair", name="cp")
                        for j in range(2):
                            bd = b0 + j
                            nc.tensor.matmul(
                                out=ps[:, j * NT : (j + 1) * NT],
                                lhsT=kb[quad][
                                    32 * bd : 32 * bd + KD,
                                    mc * 128 : (mc + 1) * 128,
                                ],
                                rhs=qb[quad][
                                    32 * bd : 32 * bd + KD,
                                    nt * NT : (nt + 1) * NT,
                                ],
                                start=True,
                                stop=True,
                                tile_position=(32 * bd, 0),
                            )
                        if prev is not None:
                            emit_pv(*prev)
                        stage_fired = False
                        if pending is not None and pending and step == pending[0][0]:
                            pending.pop(0)[1]()
                            stage_fired = True
                            if not pending:
                                pending = None
                        if not stage_fired and prologue_units and (
                            step >= 4 or (b == 0 and nt == 0)
                        ):
                            prologue_units.pop(0)()
                        p_sb = pe_pool.tile([128, 2 * NT], BF16, tag="p", name="p")
                        nc.scalar.activation(
                            out=p_sb,
                            in_=ps,
                            func=mybir.ActivationFunctionType.Exp,
                            scale=0.25,
                        )
                        pm = pm_pool.tile([128, 2 * NT], BF16, tag="pm", name="pm")
                        adj_src = adjT_sb_of(ios, b)[:, mc, nt * NT : (nt + 1) * NT]
                        adj_rep = bass.AP(
                            tensor=adj_src.tensor,
                            offset=adj_src.offset,
                            ap=[list(adj_src.ap[0]), [0, 2]]
                            + [list(a) for a in adj_src.ap[1:]],
                        )
                        nc.vector.tensor_mul(pm, p_sb, adj_rep)
                        prev = (pm, mc, pair)
                        step += 1
                emit_pv(*prev)
                pending = make_epilogue(b, nt, hp)
            del ios[b]
        for _, fn in pending:
            fn()
    return nc


def adjT_sb_of(ios, b):
    return ios[b][1]


def _split_multi_waits(nc):
    """walrus codegen in this container allows only one sync-wait per
    instruction; hoist extra waits onto preceding same-engine nops."""
    import copy
    import bass_rust

    tmpl_nc = bass.Bass()
    tmpls = {}
    for en in ["vector", "scalar", "tensor", "gpsimd", "sync"]:
        ins = getattr(tmpl_nc, en).nop().ins
        tmpls[str(ins.engine)] = ins

    uid = [0]
    for fn in nc.m.functions:
        for bb in fn.blocks:
            out = []
            for ins in bb.instructions:
                si = ins.sync_info
                waits = list(si.on_wait) if si is not None else []
                if len(waits) > 1:
                    for w in waits[:-1]:
                        nop = copy.deepcopy(tmpls[str(ins.engine)])
                        uid[0] += 1
                        nop.name = f"I-splitw-{uid[0]}"
                        nop.sync_info = bass_rust.SyncInfo(
                            on_wait=[w], on_update=[]
                        )
                        out.append(nop)
                    ins.sync_info = bass_rust.SyncInfo(
                        on_wait=[waits[-1]], on_update=list(si.on_update)
                    )
                out.append(ins)
            bb.instructions = out
    return nc


_cache = {}


def _get_nc():
    if "nc" not in _cache:
        _cache["nc"] = _split_multi_waits(build_kernel())
    return _cache["nc"]


def _prep_weights(W_query, W_key, W_val, W_out):
    bf = ml_dtypes.bfloat16
    wqb = np.zeros((2, E, 128), bf)
    wkb = np.zeros((2, E, 128), bf)
    wob = np.zeros((2, 128, E), bf)
    for h in range(H):
        q, c = h // 4, h % 4
        wqb[q, :, 32 * c : 32 * c + KD] = W_query[h].astype(bf)
        wkb[q, :, 32 * c : 32 * c + KD] = W_key[h].astype(bf)
        wob[q, 32 * c : 32 * c + KD, :] = W_out[h].astype(bf)
    wv = np.ascontiguousarray(
        np.asarray(W_val, np.float32).transpose(1, 0, 2).reshape(E, H * KD)
    ).astype(bf)
    sel = np.zeros((4, 128), np.float32)
    for c in range(4):
        sel[c, 32 * c : 32 * c + KD + 1] = 1.0
    return wqb, wkb, wv, wob, sel


def kernel(h, adj_c, W_query, W_key, W_val, W_out, trace=False):
    h = np.asarray(h, np.float32)
    adj = np.asarray(adj_c)
    bf = ml_dtypes.bfloat16
    hT = np.ascontiguousarray(h.transpose(0, 2, 1)).astype(bf)  # [B, E, N]
    adjT = np.ascontiguousarray(
        adj.transpose(0, 2, 1).astype(bf)
    )  # [B, N(m), N(n)] bf16
    wqb, wkb, wv, wob, sel = _prep_weights(
        np.asarray(W_query, np.float32),
        np.asarray(W_key, np.float32),
        np.asarray(W_val, np.float32),
        np.asarray(W_out, np.float32),
    )

    nc = _get_nc()
    in_maps = []
    for c in range(CORES):
        s = slice(c * BPC, (c + 1) * BPC)
        in_maps.append(
            {
                "ht": np.ascontiguousarray(hT[s]),
                "hn": np.ascontiguousarray(h[s]),
                "adjt": np.ascontiguousarray(adjT[s]),
                "wqb": wqb,
                "wkb": wkb,
                "wv": wv,
                "wob": wob,
                "sel": sel,
            }
        )
    res = run_bass_kernel_spmd(nc, in_maps, core_ids=list(range(CORES)), trace=trace)
    out = np.concatenate([r["out"] for r in res.results], axis=0)
    if trace:
        return out, res
    return out


# revision 40
# speedup vs baseline: 1.0480x; 1.0476x over previous
"""Multi-head graph attention (GAT-style) Trainium2 Bass kernel, v3.

Full-input contract: kernel(**inputs) takes the complete arrays, shards
batch-wise across 8 NeuronCores (2 batches each), and gathers the output.

Math per batch b, head h (KD=16 head dim):
  Q = h @ Wq_h, K = h @ Wk_h, V = h @ Wv_h            [N, 16]
  compatT[m, n] = (K Q^T)[m, n]                        [N, N] (transposed)
  p = exp(0.25 * compatT) * adjT                       (mask after exp; exact:
      masked entries are exactly 0, matching softmax(-inf) * adj)
  headsT[v, n] = (V'.T @ p)  with V' = [V | 1 | 0pad]  -> row 16 = denominator
  out[n, :] = sum_h (headsT_h / den_h).T @ Wout_h + h[n, :]

Design (v3): ACT's exp stream (~141us) is the pacing engine; everything
else hides under it.
 - Heads live in 32-partition bands (head h -> quad h//4, band h%4).
   Banded zero-padded projection weights put q/k of band b at SBUF
   partitions [32b, 32b+16), so the K=16 compat matmuls of a head PAIR
   run concurrently via PE row tiling (tile_position=(32b, 0)).
 - Each pair's two [128,512] compat tiles share one 2-bank PSUM buffer;
   one exp call covers both.  Pair buffers rotate 3-deep (banks 0-5);
   PV accumulators for the two quads hold banks 6-7 per n-half.
 - PV matmuls lag one pair behind compat in PE program order.
 - The per-n-half epilogue (hu copy, reciprocal of the denominator rows
   via a [128,16] reshape bounce through DRAM, band-broadcast, W_out
   matmul + residual) is software-pipelined INTO the next n-half's
   stream: DVE/DMA work at step 2, the PE out-matmuls at step 16, so
   the in-order PE queue never blocks the next compat matmuls.
 - Next batch's input DMAs prefetch at the previous batch's nt=0 start;
   its projection/V-build units spread one-per-step through nt=1.
"""

import os
import numpy as np
import ml_dtypes
from contextlib import ExitStack

import concourse.bass as bass
import concourse.mybir as mybir
import concourse.tile as tile
from concourse.bass_utils import run_bass_kernel_spmd

B, N, E, H, KD = 16, 1024, 128, 8, 16
CORES = 8
BPC = B // CORES  # batches per core
F32 = mybir.dt.float32
BF16 = mybir.dt.bfloat16
NT = 512  # n-half width (one PSUM bank of fp32 per [128, NT] tile)
MC = N // 128  # number of 128-row chunks of m
VP = 32  # padded per-head V columns (16 vals + 1 ones + 15 zeros)
NR = NT // 32  # free elems per lane in the [128, NR] reciprocal

EPI1_STEP = int(os.environ.get("KEPI1", "2"))
EPI2_STEP = int(os.environ.get("KEPI2", "16"))


def build_kernel():
    nc = bass.Bass()
    hT_d = nc.dram_tensor("ht", [BPC, E, N], BF16, kind="ExternalInput")
    h_d = nc.dram_tensor("hn", [BPC, N, E], F32, kind="ExternalInput")
    adjt_d = nc.dram_tensor("adjt", [BPC, N, N], BF16, kind="ExternalInput")
    wqb_d = nc.dram_tensor("wqb", [2, E, 128], BF16, kind="ExternalInput")
    wkb_d = nc.dram_tensor("wkb", [2, E, 128], BF16, kind="ExternalInput")
    wv_d = nc.dram_tensor("wv", [E, H * KD], BF16, kind="ExternalInput")
    wob_d = nc.dram_tensor("wob", [2, 128, E], BF16, kind="ExternalInput")
    sel_d = nc.dram_tensor("sel", [4, 128], BF16, kind="ExternalInput")
    out_d = nc.dram_tensor("out", [BPC, N, E], F32, kind="ExternalOutput")

    with ExitStack() as ctx:
        tc = ctx.enter_context(tile.TileContext(nc))
        consts = ctx.enter_context(tc.tile_pool(name="consts", bufs=1))
        io_pool = ctx.enter_context(tc.tile_pool(name="io", bufs=2))
        qk_pool = ctx.enter_context(tc.tile_pool(name="qk", bufs=2))
        v_pool = ctx.enter_context(tc.tile_pool(name="v", bufs=2))
        pe_pool = ctx.enter_context(tc.tile_pool(name="pe", bufs=4))
        pm_pool = ctx.enter_context(tc.tile_pool(name="pm", bufs=4))
        hn_pool = ctx.enter_context(tc.tile_pool(name="hd", bufs=2))
        dn_pool = ctx.enter_context(tc.tile_pool(name="dn", bufs=2))
        ob_pool = ctx.enter_context(tc.tile_pool(name="ob", bufs=2))
        ps_pair = ctx.enter_context(tc.tile_pool(name="ps_p", bufs=3, space="PSUM"))
        ps_hp = ctx.enter_context(tc.tile_pool(name="ps_h", bufs=1, space="PSUM"))
        dram = ctx.enter_context(tc.tile_pool(name="dram", bufs=2, space="DRAM"))

        wv_sb = consts.tile([E, H * KD], BF16, tag="wv")
        wq_q = [consts.tile([E, 128], BF16, tag=f"wq{q}", name=f"wq{q}") for q in range(2)]
        wk_q = [consts.tile([E, 128], BF16, tag=f"wk{q}", name=f"wk{q}") for q in range(2)]
        wo_q = [consts.tile([128, E], BF16, tag=f"wo{q}", name=f"wo{q}") for q in range(2)]

        def load_weights():
            for q in range(2):
                nc.sync.dma_start(out=wq_q[q], in_=wqb_d[q])
                nc.sync.dma_start(out=wk_q[q], in_=wkb_d[q])
                nc.sync.dma_start(out=wo_q[q], in_=wob_d[q])
            nc.sync.dma_start(out=wv_sb, in_=wv_d[:, :])
            nc.sync.dma_start(out=sel_sb, in_=sel_d[:, :])

        # band-broadcast selector: sel[c, 32c+j] = 1 (j < 17) so
        # (sel.T @ rec4) replicates rec4 row c into band c's rows on PE.
        sel_sb = consts.tile([4, 128], BF16, tag="sel")

        ios = {}

        def prefetch(b, mid=None):
            hT_sb = io_pool.tile([E, N], BF16, tag="ht", name="hts")
            nc.sync.dma_start(out=hT_sb, in_=hT_d[b, :, :])
            if mid is not None:
                mid()
            adjT_sb = io_pool.tile([128, MC, N], BF16, tag="adj", name="adjs")
            nc.sync.dma_start(
                out=adjT_sb, in_=adjt_d[b].rearrange("(c p) n -> p c n", p=128)
            )
            h_sb = io_pool.tile([128, MC, E], F32, tag="hn", name="hns")
            nc.sync.dma_start(
                out=h_sb, in_=h_d[b].rearrange("(c p) e -> p c e", p=128)
            )
            ios[b] = (hT_sb, adjT_sb, h_sb)

        bands = {}

        def make_prologue_units(b):
            """Projection + V-build for batch b as a list of closures,
            each one PE matmul + one DVE copy (+memsets)."""
            hT_sb = ios[b][0]
            qb = [qk_pool.tile([128, N], BF16, tag=f"q{q}", name=f"qb{q}") for q in range(2)]
            kb = [qk_pool.tile([128, N], BF16, tag=f"k{q}", name=f"kb{q}") for q in range(2)]
            v_nat = [
                v_pool.tile([128, H, VP], BF16, tag=f"v{mc}", name=f"v{mc}")
                for mc in range(MC)
            ]
            bands[b] = (qb, kb, v_nat)
            units = []

            def proj_unit(w_sb, dst, nt):
                def run():
                    ps = ps_pair.tile([128, NT], F32, tag="pair", name="pp")
                    nc.tensor.matmul(
                        out=ps,
                        lhsT=w_sb,
                        rhs=hT_sb[:, nt * NT : (nt + 1) * NT],
                        start=True,
                        stop=True,
                    )
                    nc.vector.tensor_copy(
                        out=dst[:, nt * NT : (nt + 1) * NT], in_=ps
                    )

                return run

            def v_unit(mc):
                def run():
                    v_ps = ps_pair.tile([128, H * KD], F32, tag="pair", name="vp")
                    nc.tensor.matmul(
                        out=v_ps,
                        lhsT=hT_sb[:, mc * 128 : (mc + 1) * 128],
                        rhs=wv_sb,
                        start=True,
                        stop=True,
                    )
                    vt = v_nat[mc]
                    nc.vector.tensor_copy(
                        out=vt[:, :, 0:KD],
                        in_=v_ps.rearrange("p (h k) -> p h k", k=KD),
                    )
                    nc.vector.memset(vt[:, :, KD : KD + 1], 1.0)
                    nc.vector.memset(vt[:, :, KD + 1 : VP], 0.0)

                return run

            # streaming order: first the k/q halves the next segment's early
            # steps need, then V tiles in PV-consumption order, then the
            # q n-halves only needed by the following (nt=1) segment.
            units.append(proj_unit(wk_q[0], kb[0], 0))
            units.append(proj_unit(wq_q[0], qb[0], 0))
            units.append(proj_unit(wk_q[1], kb[1], 0))
            units.append(proj_unit(wq_q[1], qb[1], 0))
            units.append(v_unit(0))
            units.append(v_unit(1))
            units.append(v_unit(2))
            units.append(proj_unit(wk_q[0], kb[0], 1))
            units.append(proj_unit(wk_q[1], kb[1], 1))
            for mc in range(3, MC):
                units.append(v_unit(mc))
            units.append(proj_unit(wq_q[0], qb[0], 1))
            units.append(proj_unit(wq_q[1], qb[1], 1))
            return units

        def make_epilogue(b, nt, hp):
            """Normalize + W_out + residual for (b, nt) as staged closures:
            each stage's DVE ops run only after their DMA deps have had a
            few pipeline steps of latency, so the in-order DVE queue never
            blocks the exp/mask stream."""
            h_sb = ios[b][2]
            hus, r128s, hn2 = [], [], []
            rec_drams = []

            rec4s, rec_ps = [], []

            def stage_a():  # hu copies + [4,NT]->[128,NR] gather DMAs
                for q in range(2):
                    hu = hn_pool.tile([128, NT], F32, tag=f"hu{q}", name=f"hu{q}")
                    nc.vector.tensor_copy(out=hu, in_=hp[q])
                    hus.append(hu)
                for q in range(2):
                    d128 = dn_pool.tile([128, NR], F32, tag=f"d128{q}", name=f"d1{q}")
                    for c in range(4):
                        src = hus[q][32 * c + KD : 32 * c + KD + 1, :]
                        src_r = bass.AP(
                            tensor=src.tensor,
                            offset=src.offset,
                            ap=[list(src.ap[0]), [NR, 32], [1, NR]],
                        )
                        nc.gpsimd.dma_start(
                            out=d128[32 * c : 32 * c + 32, :], in_=src_r
                        )
                    r128s.append(d128)

            def stage_b():  # reciprocal + scatter back to [4, NT] rows
                for q in range(2):
                    r128 = dn_pool.tile([128, NR], F32, tag=f"r128{q}", name=f"r1{q}")
                    nc.vector.reciprocal(out=r128, in_=r128s[q])
                    r128s[q] = r128
                for q in range(2):
                    r128b = dn_pool.tile([128, NR], BF16, tag=f"r128b{q}", name=f"rb{q}")
                    nc.vector.tensor_copy(out=r128b, in_=r128s[q])
                    rec4 = dn_pool.tile([4, NT], BF16, tag=f"rec4{q}", name=f"rc{q}")
                    dst = rec4[:, :]
                    dst_r = bass.AP(
                        tensor=dst.tensor,
                        offset=dst.offset,
                        ap=[list(dst.ap[0]), [NR, 32], [1, NR]],
                    )
                    nc.gpsimd.dma_start(out=dst_r, in_=r128b)
                    rec4s.append(rec4)

            def stage_c():  # selector-matmul broadcast on PE
                for q in range(2):
                    bc_ps = ps_pair.tile([128, NT], F32, tag="pair", name="bc")
                    nc.tensor.matmul(
                        out=bc_ps, lhsT=sel_sb, rhs=rec4s[q], start=True, stop=True
                    )
                    rec_ps.append(bc_ps)

            def stage_c2():  # normalize muls
                for q in range(2):
                    hn = hn_pool.tile([128, NT], BF16, tag=f"hn{q}", name=f"hn{q}")
                    nc.vector.tensor_mul(hn, hus[q], rec_ps[q])
                    hn2.append(hn)

            def stage_d():  # W_out matmuls + residual + store (paired chunks)
                for cp in range(NT // 256):
                    cc = nt * (NT // 128) + cp * 2
                    o_ps = ps_pair.tile([128, 2, E], F32, tag="pair", name="op")
                    for cl in range(2):
                        for q in range(2):
                            nc.tensor.matmul(
                                out=o_ps[:, cl, :],
                                lhsT=hn2[q][
                                    :, (cp * 2 + cl) * 128 : (cp * 2 + cl + 1) * 128
                                ],
                                rhs=wo_q[q],
                                start=(q == 0),
                                stop=(q == 1),
                            )
                    ob = ob_pool.tile([128, 2, E], F32, tag="ob", name="ob")
                    nc.vector.tensor_add(ob, o_ps, h_sb[:, cc : cc + 2, :])
                    nc.sync.dma_start(
                        out=out_d[b, cc * 128 : (cc + 2) * 128, :].rearrange(
                            "(c p) e -> p c e", p=128
                        ),
                        in_=ob,
                    )

            return [
                (1, stage_a),
                (6, stage_b),
                (11, stage_c),
                (13, stage_c2),
                (17, stage_d),
            ]

        # ---- main stream over (batch, n-half) segments ----
        prefetch(0, mid=load_weights)
        b0_units = make_prologue_units(0)
        for u in b0_units[:4]:
            u()
        pending = None
        prologue_units = b0_units[4:]
        for b in range(BPC):
            qb, kb, v_nat = bands.pop(b)
            for nt in range(N // NT):
                if nt == 0 and b + 1 < BPC:
                    prefetch(b + 1)
                if nt == 1 and b + 1 < BPC:
                    prologue_units = make_prologue_units(b + 1)
                hp = [
                    ps_hp.tile([128, NT], F32, tag=f"hp{q}", name=f"hp{q}")
                    for q in range(2)
                ]
                prev = None

                def emit_pv(pm, mc, pair):
                    for j in range(2):
                        hh = pair * 2 + j
                        c = hh % 4
                        nc.tensor.matmul(
                            out=hp[hh // 4][32 * c : 32 * c + VP, :],
                            lhsT=v_nat[mc][:, hh, :],
                            rhs=pm[:, j * NT : (j + 1) * NT],
                            start=(mc == 0),
                            stop=(mc == MC - 1),
                            tile_position=(0, 32 * c),
                        )

                step = 0
                for mc in range(MC):
                    for pair in range(4):
                        quad, b0 = pair // 2, (pair * 2) % 4
                        ps = ps_pair.tile([128, 2 * NT], F32, tag="pair", name="cp")
                        for j in range(2):
                            bd = b0 + j
                            nc.tensor.matmul(
                                out=ps[:, j * NT : (j + 1) * NT],
                                lhsT=kb[quad][
                                    32 * bd : 32 * bd + KD,
                                    mc * 128 : (mc + 1) * 128,
                                ],
                                rhs=qb[quad][
                                    32 * bd : 32 * bd + KD,
                                    nt * NT : (nt + 1) * NT,
                                ],
                                start=True,
                                stop=True,
                                tile_position=(32 * bd, 0),
                            )
                        if prev is not None:
                            emit_pv(*prev)
                        stage_fired = False
                        if pending is not None and pending and step == pending[0][0]:
                            pending.pop(0)[1]()
                            stage_fired = True
                            if not pending:
                                pending = None
                        if not stage_fired and prologue_units and (
                            step >= 4 or (b == 0 and nt == 0)
                        ):
                            prologue_units.pop(0)()
                        p_sb = pe_pool.tile([128, 2 * NT], BF16, tag="p", name="p")
                        nc.scalar.activation(
                            out=p_sb,
                            in_=ps,
                            func=mybir.ActivationFunctionType.Exp,
                            scale=0.25,
                        )
                        pm = pm_pool.tile([128, 2 * NT], BF16, tag="pm", name="pm")
                        adj_src = adjT_sb_of(ios, b)[:, mc, nt * NT : (nt + 1) * NT]
                        adj_rep = bass.AP(
                            tensor=adj_src.tensor,
                            offset=adj_src.offset,
                            ap=[list(adj_src.ap[0]), [0, 2]]
                            + [list(a) for a in adj_src.ap[1:]],
                        )
                        nc.vector.tensor_mul(pm, p_sb, adj_rep)
                        prev = (pm, mc, pair)
                        step += 1
                emit_pv(*prev)
                pending = make_epilogue(b, nt, hp)
            del ios[b]
        for _, fn in pending:
            fn()
    return nc


def adjT_sb_of(ios, b):
    return ios[b][1]


def _split_multi_waits(nc):
    """walrus codegen in this container allows only one sync-wait per
    instruction; hoist extra waits onto preceding same-engine nops."""
    import copy
    import bass_rust

    tmpl_nc = bass.Bass()
    tmpls = {}
    for en in ["vector", "scalar", "tensor", "gpsimd", "sync"]:
        ins = getattr(tmpl_nc, en).nop().ins
        tmpls[str(ins.engine)] = ins

    uid = [0]
    for fn in nc.m.functions:
        for bb in fn.blocks:
            out = []
            for ins in bb.instructions:
                si = ins.sync_info
                waits = list(si.on_wait) if si is not None else []
                if len(waits) > 1:
                    for w in waits[:-1]:
                        nop = copy.deepcopy(tmpls[str(ins.engine)])
                        uid[0] += 1
                        nop.name = f"I-splitw-{uid[0]}"
                        nop.sync_info = bass_rust.SyncInfo(
                            on_wait=[w], on_update=[]
                        )
                        out.append(nop)
                    ins.sync_info = bass_rust.SyncInfo(
                        on_wait=[waits[-1]], on_update=list(si.on_update)
                    )
                out.append(ins)
            bb.instructions = out
    return nc


_cache = {}


def _get_nc():
    if "nc" not in _cache:
        _cache["nc"] = _split_multi_waits(build_kernel())
    return _cache["nc"]


def _prep_weights(W_query, W_key, W_val, W_out):
    bf = ml_dtypes.bfloat16
    wqb = np.zeros((2, E, 128), bf)
    wkb = np.zeros((2, E, 128), bf)
    wob = np.zeros((2, 128, E), bf)
    for h in range(H):
        q, c = h // 4, h % 4
        wqb[q, :, 32 * c : 32 * c + KD] = W_query[h].astype(bf)
        wkb[q, :, 32 * c : 32 * c + KD] = W_key[h].astype(bf)
        wob[q, 32 * c : 32 * c + KD, :] = W_out[h].astype(bf)
    wv = np.ascontiguousarray(
        np.asarray(W_val, np.float32).transpose(1, 0, 2).reshape(E, H * KD)
    ).astype(bf)
    sel = np.zeros((4, 128), ml_dtypes.bfloat16)
    for c in range(4):
        sel[c, 32 * c : 32 * c + KD + 1] = 1.0
    return wqb, wkb, wv, wob, sel


def kernel(h, adj_c, W_query, W_key, W_val, W_out, trace=False):
    h = np.asarray(h, np.float32)
    adj = np.asarray(adj_c)
    bf = ml_dtypes.bfloat16
    hT = np.ascontiguousarray(h.transpose(0, 2, 1)).astype(bf)  # [B, E, N]
    adjT = np.ascontiguousarray(
        adj.transpose(0, 2, 1).astype(bf)
    )  # [B, N(m), N(n)] bf16
    wqb, wkb, wv, wob, sel = _prep_weights(
        np.asarray(W_query, np.float32),
        np.asarray(W_key, np.float32),
        np.asarray(W_val, np.float32),
        np.asarray(W_out, np.float32),
    )

    nc = _get_nc()
    in_maps = []
    for c in range(CORES):
        s = slice(c * BPC, (c + 1) * BPC)
        in_maps.append(
            {
                "ht": np.ascontiguousarray(hT[s]),
                "hn": np.ascontiguousarray(h[s]),
                "adjt": np.ascontiguousarray(adjT[s]),
                "wqb": wqb,
                "wkb": wkb,
                "wv": wv,
                "wob": wob,
                "sel": sel,
            }
        )
    res = run_bass_kernel_spmd(nc, in_maps, core_ids=list(range(CORES)), trace=trace)
    out = np.concatenate([r["out"] for r in res.results], axis=0)
    if trace:
        return out, res
    return out


# revision 43
# speedup vs baseline: 1.0848x; 1.0350x over previous
"""Multi-head graph attention (GAT-style) Trainium2 Bass kernel, v3.

Full-input contract: kernel(**inputs) takes the complete arrays, shards
batch-wise across 8 NeuronCores (2 batches each), and gathers the output.

Math per batch b, head h (KD=16 head dim):
  Q = h @ Wq_h, K = h @ Wk_h, V = h @ Wv_h            [N, 16]
  compatT[m, n] = (K Q^T)[m, n]                        [N, N] (transposed)
  p = exp(0.25 * compatT) * adjT                       (mask after exp; exact:
      masked entries are exactly 0, matching softmax(-inf) * adj)
  headsT[v, n] = (V'.T @ p)  with V' = [V | 1 | 0pad]  -> row 16 = denominator
  out[n, :] = sum_h (headsT_h / den_h).T @ Wout_h + h[n, :]

Design (v3): ACT's exp stream (~141us) is the pacing engine; everything
else hides under it.
 - Heads live in 32-partition bands (head h -> quad h//4, band h%4).
   Banded zero-padded projection weights put q/k of band b at SBUF
   partitions [32b, 32b+16), so the K=16 compat matmuls of a head PAIR
   run concurrently via PE row tiling (tile_position=(32b, 0)).
 - Each pair's two [128,512] compat tiles share one 2-bank PSUM buffer;
   one exp call covers both.  Pair buffers rotate 3-deep (banks 0-5);
   PV accumulators for the two quads hold banks 6-7 per n-half.
 - PV matmuls lag one pair behind compat in PE program order.
 - The per-n-half epilogue (hu copy, reciprocal of the denominator rows
   via a [128,16] reshape bounce through DRAM, band-broadcast, W_out
   matmul + residual) is software-pipelined INTO the next n-half's
   stream: DVE/DMA work at step 2, the PE out-matmuls at step 16, so
   the in-order PE queue never blocks the next compat matmuls.
 - Next batch's input DMAs prefetch at the previous batch's nt=0 start;
   its projection/V-build units spread one-per-step through nt=1.
"""

import os
import numpy as np
import ml_dtypes
from contextlib import ExitStack

import concourse.bass as bass
import concourse.mybir as mybir
import concourse.tile as tile
from concourse.bass_utils import run_bass_kernel_spmd

B, N, E, H, KD = 16, 1024, 128, 8, 16
CORES = 8
BPC = B // CORES  # batches per core
F32 = mybir.dt.float32
BF16 = mybir.dt.bfloat16
NT = 512  # n-half width (one PSUM bank of fp32 per [128, NT] tile)
MC = N // 128  # number of 128-row chunks of m
VP = 32  # padded per-head V columns (16 vals + 1 ones + 15 zeros)
NR = NT // 32  # free elems per lane in the [128, NR] reciprocal

EPI1_STEP = int(os.environ.get("KEPI1", "2"))
EPI2_STEP = int(os.environ.get("KEPI2", "16"))


def build_kernel():
    nc = bass.Bass()
    hT_d = nc.dram_tensor("ht", [BPC, E, N], BF16, kind="ExternalInput")
    h_d = nc.dram_tensor("hn", [BPC, N, E], F32, kind="ExternalInput")
    adjt_d = nc.dram_tensor("adjt", [BPC, N, N], BF16, kind="ExternalInput")
    wqb_d = nc.dram_tensor("wqb", [2, E, 128], BF16, kind="ExternalInput")
    wkb_d = nc.dram_tensor("wkb", [2, E, 128], BF16, kind="ExternalInput")
    wv_d = nc.dram_tensor("wv", [E, H * KD], BF16, kind="ExternalInput")
    wob_d = nc.dram_tensor("wob", [2, 128, E], BF16, kind="ExternalInput")
    sel_d = nc.dram_tensor("sel", [4, 128], BF16, kind="ExternalInput")
    out_d = nc.dram_tensor("out", [BPC, N, E], F32, kind="ExternalOutput")

    with ExitStack() as ctx:
        tc = ctx.enter_context(tile.TileContext(nc))
        consts = ctx.enter_context(tc.tile_pool(name="consts", bufs=1))
        io_pool = ctx.enter_context(tc.tile_pool(name="io", bufs=2))
        qk_pool = ctx.enter_context(tc.tile_pool(name="qk", bufs=2))
        v_pool = ctx.enter_context(tc.tile_pool(name="v", bufs=2))
        pe_pool = ctx.enter_context(tc.tile_pool(name="pe", bufs=4))
        pm_pool = ctx.enter_context(tc.tile_pool(name="pm", bufs=4))
        hn_pool = ctx.enter_context(tc.tile_pool(name="hd", bufs=2))
        dn_pool = ctx.enter_context(tc.tile_pool(name="dn", bufs=2))
        ob_pool = ctx.enter_context(tc.tile_pool(name="ob", bufs=2))
        ps_pair = ctx.enter_context(tc.tile_pool(name="ps_p", bufs=3, space="PSUM"))
        ps_hp = ctx.enter_context(tc.tile_pool(name="ps_h", bufs=1, space="PSUM"))
        dram = ctx.enter_context(tc.tile_pool(name="dram", bufs=2, space="DRAM"))

        wv_sb = consts.tile([E, H * KD], BF16, tag="wv")
        wq_q = [consts.tile([E, 128], BF16, tag=f"wq{q}", name=f"wq{q}") for q in range(2)]
        wk_q = [consts.tile([E, 128], BF16, tag=f"wk{q}", name=f"wk{q}") for q in range(2)]
        wo_q = [consts.tile([128, E], BF16, tag=f"wo{q}", name=f"wo{q}") for q in range(2)]

        def load_weights():
            for q in range(2):
                nc.sync.dma_start(out=wq_q[q], in_=wqb_d[q])
                nc.sync.dma_start(out=wk_q[q], in_=wkb_d[q])
                nc.sync.dma_start(out=wo_q[q], in_=wob_d[q])
            nc.sync.dma_start(out=wv_sb, in_=wv_d[:, :])
            nc.sync.dma_start(out=sel_sb, in_=sel_d[:, :])

        # band-broadcast selector: sel[c, 32c+j] = 1 (j < 17) so
        # (sel.T @ rec4) replicates rec4 row c into band c's rows on PE.
        sel_sb = consts.tile([4, 128], BF16, tag="sel")

        ios = {}

        def prefetch(b, mid=None):
            hT_sb = io_pool.tile([E, N], BF16, tag="ht", name="hts")
            nc.sync.dma_start(out=hT_sb, in_=hT_d[b, :, :])
            if mid is not None:
                mid()
            adjT_sb = io_pool.tile([128, MC, N], BF16, tag="adj", name="adjs")
            nc.sync.dma_start(
                out=adjT_sb, in_=adjt_d[b].rearrange("(c p) n -> p c n", p=128)
            )
            h_sb = io_pool.tile([128, MC, E], F32, tag="hn", name="hns")
            nc.sync.dma_start(
                out=h_sb, in_=h_d[b].rearrange("(c p) e -> p c e", p=128)
            )
            ios[b] = (hT_sb, adjT_sb, h_sb)

        bands = {}

        def make_prologue_units(b):
            """Projection + V-build for batch b as a list of closures,
            each one PE matmul + one DVE copy (+memsets)."""
            hT_sb = ios[b][0]
            qb = [qk_pool.tile([128, N], BF16, tag=f"q{q}", name=f"qb{q}") for q in range(2)]
            kb = [qk_pool.tile([128, N], BF16, tag=f"k{q}", name=f"kb{q}") for q in range(2)]
            v_nat = [
                v_pool.tile([128, H, VP], BF16, tag=f"v{mc}", name=f"v{mc}")
                for mc in range(MC)
            ]
            bands[b] = (qb, kb, v_nat)
            units = []

            def proj_unit(w_sb, dst, nt):
                def run():
                    ps = ps_pair.tile([128, NT], F32, tag="pair", name="pp")
                    nc.tensor.matmul(
                        out=ps,
                        lhsT=w_sb,
                        rhs=hT_sb[:, nt * NT : (nt + 1) * NT],
                        start=True,
                        stop=True,
                    )
                    nc.vector.tensor_copy(
                        out=dst[:, nt * NT : (nt + 1) * NT], in_=ps
                    )

                return run

            def v_unit(mc):
                def run():
                    v_ps = ps_pair.tile([128, H * KD], F32, tag="pair", name="vp")
                    nc.tensor.matmul(
                        out=v_ps,
                        lhsT=hT_sb[:, mc * 128 : (mc + 1) * 128],
                        rhs=wv_sb,
                        start=True,
                        stop=True,
                    )
                    vt = v_nat[mc]
                    nc.vector.tensor_copy(
                        out=vt[:, :, 0:KD],
                        in_=v_ps.rearrange("p (h k) -> p h k", k=KD),
                    )
                    nc.vector.memset(vt[:, :, KD : KD + 1], 1.0)
                    nc.vector.memset(vt[:, :, KD + 1 : VP], 0.0)

                return run

            # streaming order: first the k/q halves the next segment's early
            # steps need, then V tiles in PV-consumption order, then the
            # q n-halves only needed by the following (nt=1) segment.
            units.append(proj_unit(wk_q[0], kb[0], 0))
            units.append(proj_unit(wq_q[0], qb[0], 0))
            units.append(proj_unit(wk_q[1], kb[1], 0))
            units.append(proj_unit(wq_q[1], qb[1], 0))
            units.append(v_unit(0))
            units.append(v_unit(1))
            units.append(v_unit(2))
            units.append(proj_unit(wk_q[0], kb[0], 1))
            units.append(proj_unit(wk_q[1], kb[1], 1))
            for mc in range(3, MC):
                units.append(v_unit(mc))
            units.append(proj_unit(wq_q[0], qb[0], 1))
            units.append(proj_unit(wq_q[1], qb[1], 1))
            return units

        def make_epilogue(b, nt, hp, final=False):
            """Normalize + W_out + residual for (b, nt) as staged closures:
            each stage's DVE ops run only after their DMA deps have had a
            few pipeline steps of latency, so the in-order DVE queue never
            blocks the exp/mask stream."""
            h_sb = ios[b][2]
            hus, r128s, hn2 = [], [], []
            rec_drams = []

            rec4s, rec_ps = [], []
            # final flush: both quad chains must run concurrently, so route
            # quad 1's small DMAs through ACT, idle once the exps are done.
            qeng = [nc.gpsimd, nc.scalar if final else nc.gpsimd]

            def stage_a():  # hu copies + [4,NT]->[128,NR] gather DMAs
                for q in range(2):
                    hu = hn_pool.tile([128, NT], F32, tag=f"hu{q}", name=f"hu{q}")
                    nc.vector.tensor_copy(out=hu, in_=hp[q])
                    hus.append(hu)
                for q in range(2):
                    d128 = dn_pool.tile([128, NR], F32, tag=f"d128{q}", name=f"d1{q}")
                    for c in range(4):
                        src = hus[q][32 * c + KD : 32 * c + KD + 1, :]
                        src_r = bass.AP(
                            tensor=src.tensor,
                            offset=src.offset,
                            ap=[list(src.ap[0]), [NR, 32], [1, NR]],
                        )
                        qeng[q].dma_start(
                            out=d128[32 * c : 32 * c + 32, :], in_=src_r
                        )
                    r128s.append(d128)

            def stage_b():  # reciprocal + scatter back to [4, NT] rows
                for q in range(2):
                    r128 = dn_pool.tile([128, NR], F32, tag=f"r128{q}", name=f"r1{q}")
                    nc.vector.reciprocal(out=r128, in_=r128s[q])
                    r128s[q] = r128
                for q in range(2):
                    r128b = dn_pool.tile([128, NR], BF16, tag=f"r128b{q}", name=f"rb{q}")
                    nc.vector.tensor_copy(out=r128b, in_=r128s[q])
                    rec4 = dn_pool.tile([4, NT], BF16, tag=f"rec4{q}", name=f"rc{q}")
                    dst = rec4[:, :]
                    dst_r = bass.AP(
                        tensor=dst.tensor,
                        offset=dst.offset,
                        ap=[list(dst.ap[0]), [NR, 32], [1, NR]],
                    )
                    qeng[q].dma_start(out=dst_r, in_=r128b)
                    rec4s.append(rec4)

            def stage_c():  # selector-matmul broadcast on PE
                for q in range(2):
                    bc_ps = ps_pair.tile([128, NT], F32, tag="pair", name="bc")
                    nc.tensor.matmul(
                        out=bc_ps, lhsT=sel_sb, rhs=rec4s[q], start=True, stop=True
                    )
                    rec_ps.append(bc_ps)

            def stage_c2():  # normalize muls
                for q in range(2):
                    hn = hn_pool.tile([128, NT], BF16, tag=f"hn{q}", name=f"hn{q}")
                    nc.vector.tensor_mul(hn, hus[q], rec_ps[q])
                    hn2.append(hn)

            def stage_d():  # W_out matmuls + residual + store (paired chunks)
                for cp in range(NT // 256):
                    cc = nt * (NT // 128) + cp * 2
                    o_ps = ps_pair.tile([128, 2, E], F32, tag="pair", name="op")
                    for cl in range(2):
                        for q in range(2):
                            nc.tensor.matmul(
                                out=o_ps[:, cl, :],
                                lhsT=hn2[q][
                                    :, (cp * 2 + cl) * 128 : (cp * 2 + cl + 1) * 128
                                ],
                                rhs=wo_q[q],
                                start=(q == 0),
                                stop=(q == 1),
                            )
                    ob = ob_pool.tile([128, 2, E], F32, tag="ob", name="ob")
                    nc.vector.tensor_add(ob, o_ps, h_sb[:, cc : cc + 2, :])
                    nc.sync.dma_start(
                        out=out_d[b, cc * 128 : (cc + 2) * 128, :].rearrange(
                            "(c p) e -> p c e", p=128
                        ),
                        in_=ob,
                    )

            return [
                (1, stage_a),
                (6, stage_b),
                (11, stage_c),
                (13, stage_c2),
                (17, stage_d),
            ]

        # ---- main stream over (batch, n-half) segments ----
        prefetch(0, mid=load_weights)
        b0_units = make_prologue_units(0)
        for u in b0_units[:4]:
            u()
        pending = None
        prologue_units = b0_units[4:]
        for b in range(BPC):
            qb, kb, v_nat = bands.pop(b)
            for nt in range(N // NT):
                if nt == 0 and b + 1 < BPC:
                    prefetch(b + 1)
                if nt == 1 and b + 1 < BPC:
                    prologue_units = make_prologue_units(b + 1)
                hp = [
                    ps_hp.tile([128, NT], F32, tag=f"hp{q}", name=f"hp{q}")
                    for q in range(2)
                ]
                unit_next = [0]
                prev = None

                def emit_pv(pm, mc, pair):
                    for j in range(2):
                        hh = pair * 2 + j
                        c = hh % 4
                        nc.tensor.matmul(
                            out=hp[hh // 4][32 * c : 32 * c + VP, :],
                            lhsT=v_nat[mc][:, hh, :],
                            rhs=pm[:, j * NT : (j + 1) * NT],
                            start=(mc == 0),
                            stop=(mc == MC - 1),
                            tile_position=(0, 32 * c),
                        )

                step = 0
                for mc in range(MC):
                    for pair in range(4):
                        quad, b0 = pair // 2, (pair * 2) % 4
                        ps = ps_pair.tile([128, 2 * NT], F32, tag="pair", name="cp")
                        for j in range(2):
                            bd = b0 + j
                            nc.tensor.matmul(
                                out=ps[:, j * NT : (j + 1) * NT],
                                lhsT=kb[quad][
                                    32 * bd : 32 * bd + KD,
                                    mc * 128 : (mc + 1) * 128,
                                ],
                                rhs=qb[quad][
                                    32 * bd : 32 * bd + KD,
                                    nt * NT : (nt + 1) * NT,
                                ],
                                start=True,
                                stop=True,
                                tile_position=(32 * bd, 0),
                            )
                        if prev is not None:
                            emit_pv(*prev)
                        stage_fired = False
                        if pending is not None and pending and step == pending[0][0]:
                            pending.pop(0)[1]()
                            stage_fired = True
                            if not pending:
                                pending = None
                        if (
                            not stage_fired
                            and prologue_units
                            and (step >= 4 or (b == 0 and nt == 0))
                            and step >= unit_next[0]
                        ):
                            prologue_units.pop(0)()
                            unit_next[0] = step + 2
                        p_sb = pe_pool.tile([128, 2 * NT], BF16, tag="p", name="p")
                        nc.scalar.activation(
                            out=p_sb,
                            in_=ps,
                            func=mybir.ActivationFunctionType.Exp,
                            scale=0.25,
                        )
                        pm = pm_pool.tile([128, 2 * NT], BF16, tag="pm", name="pm")
                        adj_src = adjT_sb_of(ios, b)[:, mc, nt * NT : (nt + 1) * NT]
                        adj_rep = bass.AP(
                            tensor=adj_src.tensor,
                            offset=adj_src.offset,
                            ap=[list(adj_src.ap[0]), [0, 2]]
                            + [list(a) for a in adj_src.ap[1:]],
                        )
                        nc.vector.tensor_mul(pm, p_sb, adj_rep)
                        prev = (pm, mc, pair)
                        step += 1
                emit_pv(*prev)
                pending = make_epilogue(b, nt, hp, final=(b == BPC - 1 and nt == N // NT - 1))
            del ios[b]
        for _, fn in pending:
            fn()
    return nc


def adjT_sb_of(ios, b):
    return ios[b][1]


def _split_multi_waits(nc):
    """walrus codegen in this container allows only one sync-wait per
    instruction; hoist extra waits onto preceding same-engine nops."""
    import copy
    import bass_rust

    tmpl_nc = bass.Bass()
    tmpls = {}
    for en in ["vector", "scalar", "tensor", "gpsimd", "sync"]:
        ins = getattr(tmpl_nc, en).nop().ins
        tmpls[str(ins.engine)] = ins

    uid = [0]
    for fn in nc.m.functions:
        for bb in fn.blocks:
            out = []
            for ins in bb.instructions:
                si = ins.sync_info
                waits = list(si.on_wait) if si is not None else []
                if len(waits) > 1:
                    for w in waits[:-1]:
                        nop = copy.deepcopy(tmpls[str(ins.engine)])
                        uid[0] += 1
                        nop.name = f"I-splitw-{uid[0]}"
                        nop.sync_info = bass_rust.SyncInfo(
                            on_wait=[w], on_update=[]
                        )
                        out.append(nop)
                    ins.sync_info = bass_rust.SyncInfo(
                        on_wait=[waits[-1]], on_update=list(si.on_update)
                    )
                out.append(ins)
            bb.instructions = out
    return nc


_cache = {}


def _get_nc():
    if "nc" not in _cache:
        _cache["nc"] = _split_multi_waits(build_kernel())
    return _cache["nc"]


def _prep_weights(W_query, W_key, W_val, W_out):
    bf = ml_dtypes.bfloat16
    wqb = np.zeros((2, E, 128), bf)
    wkb = np.zeros((2, E, 128), bf)
    wob = np.zeros((2, 128, E), bf)
    for h in range(H):
        q, c = h // 4, h % 4
        wqb[q, :, 32 * c : 32 * c + KD] = W_query[h].astype(bf)
        wkb[q, :, 32 * c : 32 * c + KD] = W_key[h].astype(bf)
        wob[q, 32 * c : 32 * c + KD, :] = W_out[h].astype(bf)
    wv = np.ascontiguousarray(
        np.asarray(W_val, np.float32).transpose(1, 0, 2).reshape(E, H * KD)
    ).astype(bf)
    sel = np.zeros((4, 128), ml_dtypes.bfloat16)
    for c in range(4):
        sel[c, 32 * c : 32 * c + KD + 1] = 1.0
    return wqb, wkb, wv, wob, sel


def kernel(h, adj_c, W_query, W_key, W_val, W_out, trace=False):
    h = np.asarray(h, np.float32)
    adj = np.asarray(adj_c)
    bf = ml_dtypes.bfloat16
    hT = np.ascontiguousarray(h.transpose(0, 2, 1)).astype(bf)  # [B, E, N]
    adjT = np.ascontiguousarray(
        adj.transpose(0, 2, 1).astype(bf)
    )  # [B, N(m), N(n)] bf16
    wqb, wkb, wv, wob, sel = _prep_weights(
        np.asarray(W_query, np.float32),
        np.asarray(W_key, np.float32),
        np.asarray(W_val, np.float32),
        np.asarray(W_out, np.float32),
    )

    nc = _get_nc()
    in_maps = []
    for c in range(CORES):
        s = slice(c * BPC, (c + 1) * BPC)
        in_maps.append(
            {
                "ht": np.ascontiguousarray(hT[s]),
                "hn": np.ascontiguousarray(h[s]),
                "adjt": np.ascontiguousarray(adjT[s]),
                "wqb": wqb,
                "wkb": wkb,
                "wv": wv,
                "wob": wob,
                "sel": sel,
            }
        )
    res = run_bass_kernel_spmd(nc, in_maps, core_ids=list(range(CORES)), trace=trace)
    out = np.concatenate([r["out"] for r in res.results], axis=0)
    if trace:
        return out, res
    return out


# revision 44
# speedup vs baseline: 1.0955x; 1.0099x over previous
"""Multi-head graph attention (GAT-style) Trainium2 Bass kernel, v3.

Full-input contract: kernel(**inputs) takes the complete arrays, shards
batch-wise across 8 NeuronCores (2 batches each), and gathers the output.

Math per batch b, head h (KD=16 head dim):
  Q = h @ Wq_h, K = h @ Wk_h, V = h @ Wv_h            [N, 16]
  compatT[m, n] = (K Q^T)[m, n]                        [N, N] (transposed)
  p = exp(0.25 * compatT) * adjT                       (mask after exp; exact:
      masked entries are exactly 0, matching softmax(-inf) * adj)
  headsT[v, n] = (V'.T @ p)  with V' = [V | 1 | 0pad]  -> row 16 = denominator
  out[n, :] = sum_h (headsT_h / den_h).T @ Wout_h + h[n, :]

Design (v3): ACT's exp stream (~141us) is the pacing engine; everything
else hides under it.
 - Heads live in 32-partition bands (head h -> quad h//4, band h%4).
   Banded zero-padded projection weights put q/k of band b at SBUF
   partitions [32b, 32b+16), so the K=16 compat matmuls of a head PAIR
   run concurrently via PE row tiling (tile_position=(32b, 0)).
 - Each pair's two [128,512] compat tiles share one 2-bank PSUM buffer;
   one exp call covers both.  Pair buffers rotate 3-deep (banks 0-5);
   PV accumulators for the two quads hold banks 6-7 per n-half.
 - PV matmuls lag one pair behind compat in PE program order.
 - The per-n-half epilogue (hu copy, reciprocal of the denominator rows
   via a [128,16] reshape bounce through DRAM, band-broadcast, W_out
   matmul + residual) is software-pipelined INTO the next n-half's
   stream: DVE/DMA work at step 2, the PE out-matmuls at step 16, so
   the in-order PE queue never blocks the next compat matmuls.
 - Next batch's input DMAs prefetch at the previous batch's nt=0 start;
   its projection/V-build units spread one-per-step through nt=1.
"""

import os
import numpy as np
import ml_dtypes
from contextlib import ExitStack

import concourse.bass as bass
import concourse.mybir as mybir
import concourse.tile as tile
from concourse.bass_utils import run_bass_kernel_spmd

B, N, E, H, KD = 16, 1024, 128, 8, 16
CORES = 8
BPC = B // CORES  # batches per core
F32 = mybir.dt.float32
BF16 = mybir.dt.bfloat16
NT = 512  # n-half width (one PSUM bank of fp32 per [128, NT] tile)
MC = N // 128  # number of 128-row chunks of m
VP = 32  # padded per-head V columns (16 vals + 1 ones + 15 zeros)
NR = NT // 32  # free elems per lane in the [128, NR] reciprocal

EPI1_STEP = int(os.environ.get("KEPI1", "2"))
EPI2_STEP = int(os.environ.get("KEPI2", "16"))


def build_kernel():
    nc = bass.Bass()
    hT_d = nc.dram_tensor("ht", [BPC, E, N], BF16, kind="ExternalInput")
    h_d = nc.dram_tensor("hn", [BPC, 128, MC, E], F32, kind="ExternalInput")
    adjt_d = nc.dram_tensor("adjt", [BPC, 128, MC, N], BF16, kind="ExternalInput")
    wqb_d = nc.dram_tensor("wqb", [2, E, 128], BF16, kind="ExternalInput")
    wkb_d = nc.dram_tensor("wkb", [2, E, 128], BF16, kind="ExternalInput")
    wv_d = nc.dram_tensor("wv", [E, H * KD], BF16, kind="ExternalInput")
    wob_d = nc.dram_tensor("wob", [2, 128, E], BF16, kind="ExternalInput")
    sel_d = nc.dram_tensor("sel", [4, 128], BF16, kind="ExternalInput")
    out_d = nc.dram_tensor("out", [BPC, N, E], F32, kind="ExternalOutput")

    with ExitStack() as ctx:
        tc = ctx.enter_context(tile.TileContext(nc))
        consts = ctx.enter_context(tc.tile_pool(name="consts", bufs=1))
        io_pool = ctx.enter_context(tc.tile_pool(name="io", bufs=2))
        qk_pool = ctx.enter_context(tc.tile_pool(name="qk", bufs=2))
        v_pool = ctx.enter_context(tc.tile_pool(name="v", bufs=2))
        pe_pool = ctx.enter_context(tc.tile_pool(name="pe", bufs=4))
        pm_pool = ctx.enter_context(tc.tile_pool(name="pm", bufs=4))
        hn_pool = ctx.enter_context(tc.tile_pool(name="hd", bufs=2))
        dn_pool = ctx.enter_context(tc.tile_pool(name="dn", bufs=2))
        ob_pool = ctx.enter_context(tc.tile_pool(name="ob", bufs=2))
        ps_pair = ctx.enter_context(tc.tile_pool(name="ps_p", bufs=3, space="PSUM"))
        ps_hp = ctx.enter_context(tc.tile_pool(name="ps_h", bufs=1, space="PSUM"))
        dram = ctx.enter_context(tc.tile_pool(name="dram", bufs=2, space="DRAM"))

        wv_sb = consts.tile([E, H * KD], BF16, tag="wv")
        wq_q = [consts.tile([E, 128], BF16, tag=f"wq{q}", name=f"wq{q}") for q in range(2)]
        wk_q = [consts.tile([E, 128], BF16, tag=f"wk{q}", name=f"wk{q}") for q in range(2)]
        wo_q = [consts.tile([128, E], BF16, tag=f"wo{q}", name=f"wo{q}") for q in range(2)]

        def load_weights():
            for q in range(2):
                nc.sync.dma_start(out=wq_q[q], in_=wqb_d[q])
                nc.sync.dma_start(out=wk_q[q], in_=wkb_d[q])
                nc.sync.dma_start(out=wo_q[q], in_=wob_d[q])
            nc.sync.dma_start(out=wv_sb, in_=wv_d[:, :])
            nc.sync.dma_start(out=sel_sb, in_=sel_d[:, :])

        # band-broadcast selector: sel[c, 32c+j] = 1 (j < 17) so
        # (sel.T @ rec4) replicates rec4 row c into band c's rows on PE.
        sel_sb = consts.tile([4, 128], BF16, tag="sel")

        ios = {}

        def prefetch(b, mid=None):
            hT_sb = io_pool.tile([E, N], BF16, tag="ht", name="hts")
            nc.sync.dma_start(out=hT_sb, in_=hT_d[b, :, :])
            if mid is not None:
                mid()
            adjT_sb = io_pool.tile([128, MC, N], BF16, tag="adj", name="adjs")
            nc.sync.dma_start(out=adjT_sb, in_=adjt_d[b])
            h_sb = io_pool.tile([128, MC, E], F32, tag="hn", name="hns")
            nc.sync.dma_start(out=h_sb, in_=h_d[b])
            ios[b] = (hT_sb, adjT_sb, h_sb)

        bands = {}

        def make_prologue_units(b):
            """Projection + V-build for batch b as a list of closures,
            each one PE matmul + one DVE copy (+memsets)."""
            hT_sb = ios[b][0]
            qb = [qk_pool.tile([128, N], BF16, tag=f"q{q}", name=f"qb{q}") for q in range(2)]
            kb = [qk_pool.tile([128, N], BF16, tag=f"k{q}", name=f"kb{q}") for q in range(2)]
            v_nat = [
                v_pool.tile([128, H, VP], BF16, tag=f"v{mc}", name=f"v{mc}")
                for mc in range(MC)
            ]
            bands[b] = (qb, kb, v_nat)
            units = []

            def proj_unit(w_sb, dst, nt):
                def run():
                    ps = ps_pair.tile([128, NT], F32, tag="pair", name="pp")
                    nc.tensor.matmul(
                        out=ps,
                        lhsT=w_sb,
                        rhs=hT_sb[:, nt * NT : (nt + 1) * NT],
                        start=True,
                        stop=True,
                    )
                    nc.vector.tensor_copy(
                        out=dst[:, nt * NT : (nt + 1) * NT], in_=ps
                    )

                return run

            def v_unit(mc):
                def run():
                    v_ps = ps_pair.tile([128, H * KD], F32, tag="pair", name="vp")
                    nc.tensor.matmul(
                        out=v_ps,
                        lhsT=hT_sb[:, mc * 128 : (mc + 1) * 128],
                        rhs=wv_sb,
                        start=True,
                        stop=True,
                    )
                    vt = v_nat[mc]
                    nc.vector.tensor_copy(
                        out=vt[:, :, 0:KD],
                        in_=v_ps.rearrange("p (h k) -> p h k", k=KD),
                    )
                    nc.vector.memset(vt[:, :, KD : KD + 1], 1.0)
                    nc.vector.memset(vt[:, :, KD + 1 : VP], 0.0)

                return run

            # streaming order: first the k/q halves the next segment's early
            # steps need, then V tiles in PV-consumption order, then the
            # q n-halves only needed by the following (nt=1) segment.
            units.append(proj_unit(wk_q[0], kb[0], 0))
            units.append(proj_unit(wq_q[0], qb[0], 0))
            units.append(proj_unit(wk_q[1], kb[1], 0))
            units.append(proj_unit(wq_q[1], qb[1], 0))
            units.append(v_unit(0))
            units.append(v_unit(1))
            units.append(v_unit(2))
            units.append(proj_unit(wk_q[0], kb[0], 1))
            units.append(proj_unit(wk_q[1], kb[1], 1))
            for mc in range(3, MC):
                units.append(v_unit(mc))
            units.append(proj_unit(wq_q[0], qb[0], 1))
            units.append(proj_unit(wq_q[1], qb[1], 1))
            return units

        def make_epilogue(b, nt, hp, final=False):
            """Normalize + W_out + residual for (b, nt) as staged closures:
            each stage's DVE ops run only after their DMA deps have had a
            few pipeline steps of latency, so the in-order DVE queue never
            blocks the exp/mask stream."""
            h_sb = ios[b][2]
            hus, r128s, hn2 = [], [], []
            rec_drams = []

            rec4s, rec_ps = [], []
            # final flush: both quad chains must run concurrently, so route
            # quad 1's small DMAs through ACT, idle once the exps are done.
            qeng = [nc.gpsimd, nc.scalar if final else nc.gpsimd]

            def stage_a():  # hu copies + [4,NT]->[128,NR] gather DMAs
                for q in range(2):
                    hu = hn_pool.tile([128, NT], F32, tag=f"hu{q}", name=f"hu{q}")
                    nc.vector.tensor_copy(out=hu, in_=hp[q])
                    hus.append(hu)
                for q in range(2):
                    d128 = dn_pool.tile([128, NR], F32, tag=f"d128{q}", name=f"d1{q}")
                    for c in range(4):
                        src = hus[q][32 * c + KD : 32 * c + KD + 1, :]
                        src_r = bass.AP(
                            tensor=src.tensor,
                            offset=src.offset,
                            ap=[list(src.ap[0]), [NR, 32], [1, NR]],
                        )
                        qeng[q].dma_start(
                            out=d128[32 * c : 32 * c + 32, :], in_=src_r
                        )
                    r128s.append(d128)

            def stage_b():  # reciprocal + scatter back to [4, NT] rows
                for q in range(2):
                    r128 = dn_pool.tile([128, NR], F32, tag=f"r128{q}", name=f"r1{q}")
                    nc.vector.reciprocal(out=r128, in_=r128s[q])
                    r128s[q] = r128
                for q in range(2):
                    r128b = dn_pool.tile([128, NR], BF16, tag=f"r128b{q}", name=f"rb{q}")
                    nc.vector.tensor_copy(out=r128b, in_=r128s[q])
                    rec4 = dn_pool.tile([4, NT], BF16, tag=f"rec4{q}", name=f"rc{q}")
                    dst = rec4[:, :]
                    dst_r = bass.AP(
                        tensor=dst.tensor,
                        offset=dst.offset,
                        ap=[list(dst.ap[0]), [NR, 32], [1, NR]],
                    )
                    qeng[q].dma_start(out=dst_r, in_=r128b)
                    rec4s.append(rec4)

            def stage_c():  # selector-matmul broadcast on PE
                for q in range(2):
                    bc_ps = ps_pair.tile([128, NT], F32, tag="pair", name="bc")
                    nc.tensor.matmul(
                        out=bc_ps, lhsT=sel_sb, rhs=rec4s[q], start=True, stop=True
                    )
                    rec_ps.append(bc_ps)

            def stage_c2():  # normalize muls
                for q in range(2):
                    hn = hn_pool.tile([128, NT], BF16, tag=f"hn{q}", name=f"hn{q}")
                    nc.vector.tensor_mul(hn, hus[q], rec_ps[q])
                    hn2.append(hn)

            def stage_d():  # W_out matmuls + residual + store (paired chunks)
                for cp in range(NT // 256):
                    cc = nt * (NT // 128) + cp * 2
                    o_ps = ps_pair.tile([128, 2, E], F32, tag="pair", name="op")
                    for cl in range(2):
                        for q in range(2):
                            nc.tensor.matmul(
                                out=o_ps[:, cl, :],
                                lhsT=hn2[q][
                                    :, (cp * 2 + cl) * 128 : (cp * 2 + cl + 1) * 128
                                ],
                                rhs=wo_q[q],
                                start=(q == 0),
                                stop=(q == 1),
                            )
                    ob = ob_pool.tile([128, 2, E], F32, tag="ob", name="ob")
                    nc.vector.tensor_add(ob, o_ps, h_sb[:, cc : cc + 2, :])
                    nc.sync.dma_start(
                        out=out_d[b, cc * 128 : (cc + 2) * 128, :].rearrange(
                            "(c p) e -> p c e", p=128
                        ),
                        in_=ob,
                    )

            return [
                (1, stage_a),
                (6, stage_b),
                (11, stage_c),
                (13, stage_c2),
                (17, stage_d),
            ]

        # ---- main stream over (batch, n-half) segments ----
        prefetch(0, mid=load_weights)
        b0_units = make_prologue_units(0)
        for u in b0_units[:4]:
            u()
        pending = None
        prologue_units = b0_units[4:]
        for b in range(BPC):
            qb, kb, v_nat = bands.pop(b)
            for nt in range(N // NT):
                if nt == 0 and b + 1 < BPC:
                    prefetch(b + 1)
                if nt == 1 and b + 1 < BPC:
                    prologue_units = make_prologue_units(b + 1)
                hp = [
                    ps_hp.tile([128, NT], F32, tag=f"hp{q}", name=f"hp{q}")
                    for q in range(2)
                ]
                unit_next = [0]
                prev = None

                def emit_pv(pm, mc, pair):
                    for j in range(2):
                        hh = pair * 2 + j
                        c = hh % 4
                        nc.tensor.matmul(
                            out=hp[hh // 4][32 * c : 32 * c + VP, :],
                            lhsT=v_nat[mc][:, hh, :],
                            rhs=pm[:, j * NT : (j + 1) * NT],
                            start=(mc == 0),
                            stop=(mc == MC - 1),
                            tile_position=(0, 32 * c),
                        )

                step = 0
                for mc in range(MC):
                    for pair in range(4):
                        quad, b0 = pair // 2, (pair * 2) % 4
                        ps = ps_pair.tile([128, 2 * NT], F32, tag="pair", name="cp")
                        for j in range(2):
                            bd = b0 + j
                            nc.tensor.matmul(
                                out=ps[:, j * NT : (j + 1) * NT],
                                lhsT=kb[quad][
                                    32 * bd : 32 * bd + KD,
                                    mc * 128 : (mc + 1) * 128,
                                ],
                                rhs=qb[quad][
                                    32 * bd : 32 * bd + KD,
                                    nt * NT : (nt + 1) * NT,
                                ],
                                start=True,
                                stop=True,
                                tile_position=(32 * bd, 0),
                            )
                        if prev is not None:
                            emit_pv(*prev)
                        stage_fired = False
                        if pending is not None and pending and step == pending[0][0]:
                            pending.pop(0)[1]()
                            stage_fired = True
                            if not pending:
                                pending = None
                        if (
                            not stage_fired
                            and prologue_units
                            and (step >= 4 or (b == 0 and nt == 0))
                            and step >= unit_next[0]
                        ):
                            prologue_units.pop(0)()
                            unit_next[0] = step + 2
                        p_sb = pe_pool.tile([128, 2 * NT], BF16, tag="p", name="p")
                        nc.scalar.activation(
                            out=p_sb,
                            in_=ps,
                            func=mybir.ActivationFunctionType.Exp,
                            scale=0.25,
                        )
                        pm = pm_pool.tile([128, 2 * NT], BF16, tag="pm", name="pm")
                        adj_src = adjT_sb_of(ios, b)[:, mc, nt * NT : (nt + 1) * NT]
                        adj_rep = bass.AP(
                            tensor=adj_src.tensor,
                            offset=adj_src.offset,
                            ap=[list(adj_src.ap[0]), [0, 2]]
                            + [list(a) for a in adj_src.ap[1:]],
                        )
                        nc.vector.tensor_mul(pm, p_sb, adj_rep)
                        prev = (pm, mc, pair)
                        step += 1
                emit_pv(*prev)
                pending = make_epilogue(b, nt, hp, final=(b == BPC - 1 and nt == N // NT - 1))
            del ios[b]
        for _, fn in pending:
            fn()
    return nc


def adjT_sb_of(ios, b):
    return ios[b][1]


def _split_multi_waits(nc):
    """walrus codegen in this container allows only one sync-wait per
    instruction; hoist extra waits onto preceding same-engine nops."""
    import copy
    import bass_rust

    tmpl_nc = bass.Bass()
    tmpls = {}
    for en in ["vector", "scalar", "tensor", "gpsimd", "sync"]:
        ins = getattr(tmpl_nc, en).nop().ins
        tmpls[str(ins.engine)] = ins

    uid = [0]
    for fn in nc.m.functions:
        for bb in fn.blocks:
            out = []
            for ins in bb.instructions:
                si = ins.sync_info
                waits = list(si.on_wait) if si is not None else []
                if len(waits) > 1:
                    for w in waits[:-1]:
                        nop = copy.deepcopy(tmpls[str(ins.engine)])
                        uid[0] += 1
                        nop.name = f"I-splitw-{uid[0]}"
                        nop.sync_info = bass_rust.SyncInfo(
                            on_wait=[w], on_update=[]
                        )
                        out.append(nop)
                    ins.sync_info = bass_rust.SyncInfo(
                        on_wait=[waits[-1]], on_update=list(si.on_update)
                    )
                out.append(ins)
            bb.instructions = out
    return nc


_cache = {}


def _get_nc():
    if "nc" not in _cache:
        _cache["nc"] = _split_multi_waits(build_kernel())
    return _cache["nc"]


def _prep_weights(W_query, W_key, W_val, W_out):
    bf = ml_dtypes.bfloat16
    wqb = np.zeros((2, E, 128), bf)
    wkb = np.zeros((2, E, 128), bf)
    wob = np.zeros((2, 128, E), bf)
    for h in range(H):
        q, c = h // 4, h % 4
        wqb[q, :, 32 * c : 32 * c + KD] = W_query[h].astype(bf)
        wkb[q, :, 32 * c : 32 * c + KD] = W_key[h].astype(bf)
        wob[q, 32 * c : 32 * c + KD, :] = W_out[h].astype(bf)
    wv = np.ascontiguousarray(
        np.asarray(W_val, np.float32).transpose(1, 0, 2).reshape(E, H * KD)
    ).astype(bf)
    sel = np.zeros((4, 128), ml_dtypes.bfloat16)
    for c in range(4):
        sel[c, 32 * c : 32 * c + KD + 1] = 1.0
    return wqb, wkb, wv, wob, sel


def kernel(h, adj_c, W_query, W_key, W_val, W_out, trace=False):
    h = np.asarray(h, np.float32)
    adj = np.asarray(adj_c)
    bf = ml_dtypes.bfloat16
    hT = np.ascontiguousarray(h.transpose(0, 2, 1)).astype(bf)  # [B, E, N]
    # partition-contiguous layouts: one DMA descriptor per partition
    adjT = np.ascontiguousarray(
        adj.transpose(0, 2, 1).astype(bf).reshape(B, MC, 128, N).transpose(0, 2, 1, 3)
    )  # [B, 128, MC, N] bf16
    h_r = np.ascontiguousarray(
        h.reshape(B, MC, 128, E).transpose(0, 2, 1, 3)
    )  # [B, 128, MC, E]
    wqb, wkb, wv, wob, sel = _prep_weights(
        np.asarray(W_query, np.float32),
        np.asarray(W_key, np.float32),
        np.asarray(W_val, np.float32),
        np.asarray(W_out, np.float32),
    )

    nc = _get_nc()
    in_maps = []
    for c in range(CORES):
        s = slice(c * BPC, (c + 1) * BPC)
        in_maps.append(
            {
                "ht": np.ascontiguousarray(hT[s]),
                "hn": np.ascontiguousarray(h_r[s]),
                "adjt": np.ascontiguousarray(adjT[s]),
                "wqb": wqb,
                "wkb": wkb,
                "wv": wv,
                "wob": wob,
                "sel": sel,
            }
        )
    res = run_bass_kernel_spmd(nc, in_maps, core_ids=list(range(CORES)), trace=trace)
    out = np.concatenate([r["out"] for r in res.results], axis=0)
    if trace:
        return out, res
    return out
